# revision 6
# baseline (speedup 1.0000x reference)
"""Self-contained Trainium2 Bass kernel for 4-layer GraphSAGE (nn_LASAGE).

Strategy:
  - Nodes dst-sharded across 8 cores (6250/core, padded to 6272 = 49 blocks of 128).
  - Aggregation is done POST-matmul: agg(x)@Wl == agg(x@Wl), so per layer each
    core computes y = h @ Wl for its own shard, an AllGather replicates the full
    Y table [50176, d] to every core's DRAM, and edges gather y[src] rows with
    dma_gather (int16 idx -> table split in two halves, 4 SWDGE queues).
  - Scatter-add into dst blocks via weighted-one-hot matmuls on the PE:
    psumT[feat, dst] += G[e, feat] contracted with onehot[e, dst], where
    onehot[e, dst] = (iota == dstcol[e]) * invdeg[dst[e]] is built in a single
    DVE tensor_scalar op. The Wr-side and bias (K=1 matmul vs ones) accumulate
    into the same PSUM, so the epilogue is a single fused relu.
  - Layer1 fuses conv0+conv1 (concat -> 256 feat). Layer3 (output, d=64) uses
    non-transposed psum so rows DMA straight to the output.
"""
import sys, os, types

sys.path.insert(0, "/opt/trn_rl_repo")
import numpy as np

N = 50000
E = 800000
NCORES = 8
S = N // NCORES            # 6250 real nodes per core
SP = 6272                  # padded (49 blocks of 128)
NBLK = SP // 128
HALF = NCORES * SP // 2    # 25088 table split for int16 indices
D1 = 256                   # concat(h0, h1)
DM = 256
DO = 64
MAXI = 512                 # max idxs per dma_gather call
TPC = MAXI // 128          # tiles per full call


def _install_hooks():
    """antenv.axon_hooks shim so trace=True works in this image (optional)."""
    try:
        import antenv
        if "antenv.axon_hooks" not in sys.modules:
            mod = types.ModuleType("antenv.axon_hooks")
            mod._hook = None
            mod.set_axon_ntff_profile_hook = lambda h: setattr(mod, "_hook", h)
            mod.get_axon_ntff_profile_hook = lambda: mod._hook
            sys.modules["antenv.axon_hooks"] = mod
            antenv.axon_hooks = mod
        from antenv.axon_hooks import get_axon_ntff_profile_hook, set_axon_ntff_profile_hook
        if get_axon_ntff_profile_hook() is None:
            from trn_agent_boot.trn_boot import _ntff_profile_via_ctypes
            set_axon_ntff_profile_hook(_ntff_profile_via_ctypes("/opt/axon/libaxon_pjrt.so"))
        import concourse.bass_utils as bu
        bu.upload_artifacts = lambda tmpdir: f"file://{tmpdir}"
    except Exception:
        pass


def _preprocess(edge_index):
    """Edge lists per core, grouped by (dst block, src half), padded per-tile."""
    src = np.asarray(edge_index[0], np.int64)
    dst = np.asarray(edge_index[1], np.int64)
    core = dst // S
    dl = (dst % S).astype(np.int64)
    blk = dl // 128
    col = dl % 128
    grow = (src // S) * SP + (src % S)          # padded global table row
    half = (grow >= HALF).astype(np.int64)

    deg = np.bincount(core * S + dl, minlength=N).reshape(NCORES, S)

    order = np.lexsort((grow, blk, half, core))
    core_s, half_s, blk_s, col_s, row_s = (core[order], half[order], blk[order],
                                           col[order], grow[order])

    key = (core_s * 2 + half_s) * NBLK + blk_s
    counts = np.bincount(key, minlength=NCORES * 2 * NBLK).reshape(NCORES, 2, NBLK)
    tiles_hb = np.ceil(counts.max(axis=0) / 128).astype(np.int64)   # [2, NBLK]
    tiles_hb = np.maximum(tiles_hb, 1)

    pad_hb = tiles_hb * 128
    tot_h = pad_hb.sum(axis=1)
    seg_off = np.zeros((2, NBLK), np.int64)
    seg_off[:, 1:] = np.cumsum(pad_hb, axis=1)[:, :-1]

    srcpad = np.zeros((NCORES, 2), dtype=object)
    colpad = np.zeros((NCORES, 2), dtype=object)
    for c in range(NCORES):
        for h in range(2):
            srcpad[c, h] = np.zeros(int(tot_h[h]), np.int64)
            colpad[c, h] = np.full(int(tot_h[h]), -1, np.int64)
    grp = key
    first = np.r_[True, grp[1:] != grp[:-1]]
    gidx = np.arange(len(grp)) - np.maximum.accumulate(np.where(first, np.arange(len(grp)), 0))
    pos = seg_off[half_s, blk_s] + gidx
    for c in range(NCORES):
        m = core_s == c
        for h in range(2):
            mh = m & (half_s == h)
            p = pos[mh]
            srcpad[c, h][p] = row_s[mh] - (HALF if h else 0)
            colpad[c, h][p] = col_s[mh]

    return {
        "tiles_hb": tiles_hb, "seg_off": seg_off,
        "srcpad": srcpad, "colpad": colpad, "deg": deg,
    }


def _build_callplan(tiles_hb):
    """Gather call plan (compile-time, same for every core)."""
    calls = []
    block_tiles = {b: [] for b in range(NBLK)}
    tile_ctr = [0, 0]
    ht_tiles = [int(tiles_hb[0].sum()), int(tiles_hb[1].sum())]
    for b in range(NBLK):
        for h in range(2):
            nt = int(tiles_hb[h, b])
            done = 0
            while done < nt:
                k = min(TPC, nt - done)
                ci = len(calls)
                calls.append(dict(h=h, k=k, tile_base=tile_ctr[h], blk=b))
                for j in range(k):
                    dcol = (0 if h == 0 else ht_tiles[0]) + tile_ctr[h] + j
                    block_tiles[b].append((ci, j, dcol))
                tile_ctr[h] += k
                done += k
    return calls, block_tiles, ht_tiles


def _idx_arrays(pre, calls, core, invd_local):
    """int16 idx image [128, ncalls*32], dstloc & dstw [128, ntiles] f32."""
    ncalls = len(calls)
    idx_img = np.zeros((16, ncalls * 32), np.int16)
    tiles_total = int(pre["tiles_hb"].sum())
    dstloc = np.full((128, tiles_total), -1.0, np.float32)
    dstw = np.zeros((128, tiles_total), np.float32)
    ht0 = int(pre["tiles_hb"][0].sum())
    for ci, cl in enumerate(calls):
        h, k, tb, b = cl["h"], cl["k"], cl["tile_base"], cl["blk"]
        e0 = int(pre["seg_off"][h, b]) + (tb - int(pre["tiles_hb"][h, :b].sum())) * 128
        nidx = k * 128
        seg_src = pre["srcpad"][core, h][e0:e0 + nidx]
        seg_col = pre["colpad"][core, h][e0:e0 + nidx]
        idx_img[:, ci * 32: ci * 32 + (nidx // 16)] = seg_src.reshape(-1, 16).T.astype(np.int16)
        for t in range(k):
            dcol = (0 if h == 0 else ht0) + tb + t
            cc = seg_col[t * 128:(t + 1) * 128]
            dstloc[:, dcol] = cc
            w = np.where(cc >= 0, invd_local[np.clip(b * 128 + cc, 0, S - 1)], 0.0)
            dstw[:, dcol] = w.astype(np.float32)
    return np.tile(idx_img, (8, 1)), dstloc, dstw


def _build_bass(pre, calls, block_tiles, ht_tiles, ncalls_cols, tiles_total):
    import concourse.bass as bass
    import concourse.bacc as bacc
    import concourse.mybir as mybir
    import concourse.tile as tile

    FP32 = mybir.dt.float32
    I16 = mybir.dt.int16
    AL = mybir.AluOpType
    AF = mybir.ActivationFunctionType

    nc = bacc.Bacc("TRN2", target_bir_lowering=False, debug=False,
                   enable_asserts=False, num_devices=NCORES, num_swdge_queues=4)

    x0T = nc.dram_tensor("x0T", [128, SP], FP32, kind="ExternalInput")
    x1T = nc.dram_tensor("x1T", [128, SP], FP32, kind="ExternalInput")
    wl0 = nc.dram_tensor("wl0", [128, 128], FP32, kind="ExternalInput")
    wr0 = nc.dram_tensor("wr0", [128, 128], FP32, kind="ExternalInput")
    wl1 = nc.dram_tensor("wl1", [128, 128], FP32, kind="ExternalInput")
    wr1 = nc.dram_tensor("wr1", [128, 128], FP32, kind="ExternalInput")
    wlm = nc.dram_tensor("wlm", [256, 256], FP32, kind="ExternalInput")
    wrm = nc.dram_tensor("wrm", [256, 256], FP32, kind="ExternalInput")
    wlo = nc.dram_tensor("wlo", [256, 64], FP32, kind="ExternalInput")
    wro = nc.dram_tensor("wro", [256, 64], FP32, kind="ExternalInput")
    b01d = nc.dram_tensor("b01", [1, 256], FP32, kind="ExternalInput")
    bmd = nc.dram_tensor("bm", [1, 256], FP32, kind="ExternalInput")
    bod = nc.dram_tensor("bo", [1, 64], FP32, kind="ExternalInput")
    idxd = nc.dram_tensor("idx", [128, ncalls_cols], I16, kind="ExternalInput")
    dstld = nc.dram_tensor("dstl", [128, tiles_total], FP32, kind="ExternalInput")
    dstwd = nc.dram_tensor("dstw", [128, tiles_total], FP32, kind="ExternalInput")
    outd = nc.dram_tensor("out", [S, DO], FP32, kind="ExternalOutput")

    with tile.TileContext(nc) as tc:
        with (
            tc.tile_pool(name="const", bufs=1) as cp,
            tc.tile_pool(name="acts", bufs=1) as hp,
            tc.tile_pool(name="g", bufs=6) as gp,
            tc.tile_pool(name="oh", bufs=4) as ohp,
            tc.tile_pool(name="xs", bufs=2) as xsp,
            # PSUM budget (8 banks): ps0(2) + ps1(2) + py(2) = 6 banks
            tc.tile_pool(name="ps", bufs=2, space="PSUM") as psp,
            tc.tile_pool(name="psy", bufs=2, space="PSUM") as psyp,
            tc.tile_pool(name="ev", bufs=2) as evp,
            tc.tile_pool(name="dram", bufs=1, space="DRAM") as dp,
        ):
            def load(name, dt_, shape, src):
                t = cp.tile(shape, dt_, name=name)
                nc.sync.dma_start(out=t[:], in_=src)
                return t

            wl0t = load("wl0t", FP32, [128, 128], wl0[:])
            wr0t = load("wr0t", FP32, [128, 128], wr0[:])
            wl1t = load("wl1t", FP32, [128, 128], wl1[:])
            wr1t = load("wr1t", FP32, [128, 128], wr1[:])
            wlmt = [load(f"wlmt{i}", FP32, [128, 256], wlm[i * 128:(i + 1) * 128, :]) for i in range(2)]
            wrmt = [load(f"wrmt{i}", FP32, [128, 256], wrm[i * 128:(i + 1) * 128, :]) for i in range(2)]
            wlot = [load(f"wlot{i}", FP32, [128, 64], wlo[i * 128:(i + 1) * 128, :]) for i in range(2)]
            wrot = [load(f"wrot{i}", FP32, [128, 64], wro[i * 128:(i + 1) * 128, :]) for i in range(2)]
            b01t = load("b01t", FP32, [1, 256], b01d[:])
            bmt = load("bmt", FP32, [1, 256], bmd[:])
            bot = load("bot", FP32, [1, 64], bod[:])
            idxt = load("idxt", I16, [128, ncalls_cols], idxd[:])
            dstl = load("dstlt", FP32, [128, tiles_total], dstld[:])
            dstw = load("dstwt", FP32, [128, tiles_total], dstwd[:])

            ones_r = cp.tile([1, 128], FP32, name="ones_r")
            nc.vector.memset(ones_r[:], 1.0)
            iota_i = cp.tile([128, 128], mybir.dt.int32, name="iota_i")
            nc.gpsimd.iota(iota_i[:], pattern=[[1, 128]], base=0, channel_multiplier=0)
            iota_f = cp.tile([128, 128], FP32, name="iota_f")
            nc.vector.tensor_copy(out=iota_f[:], in_=iota_i[:])

            hT = [hp.tile([128, SP], FP32, name=f"hT{i}") for i in range(2)]
            h2T = [hp.tile([128, SP], FP32, name=f"h2T{i}") for i in range(2)]

            shared = "Shared" if NCORES > 4 else "Local"
            y01_own = dp.tile([SP, D1], FP32, name="y01_own")
            Y01 = dp.tile([NCORES * SP, D1], FP32, name="Y01", addr_space=shared)
            ym_own = dp.tile([SP, DM], FP32, name="ym_own")
            Ym = dp.tile([NCORES * SP, DM], FP32, name="Ym", addr_space=shared)
            yo_own = dp.tile([SP, DO], FP32, name="yo_own")
            Yo = dp.tile([NCORES * SP, DO], FP32, name="Yo", addr_space=shared)

            RG = [list(range(NCORES))]

            def blk_sl(b):
                return slice(b * 128, (b + 1) * 128)

            def onehot(dcol):
                oh = ohp.tile([128, 128], FP32, name="oh", tag="oh")
                nc.vector.tensor_scalar(
                    out=oh[:], in0=iota_f[:],
                    scalar1=dstl[:, dcol:dcol + 1], scalar2=dstw[:, dcol:dcol + 1],
                    op0=AL.is_equal, op1=AL.mult)
                return oh

            # ================= L1 pre: y01_own = [x0@Wl0 | x1@Wl1] =========
            for b in range(NBLK):
                x0b = xsp.tile([128, 128], FP32, name="x0b", tag="x0b")
                nc.sync.dma_start(out=x0b[:], in_=x0T[:, blk_sl(b)])
                x1b = xsp.tile([128, 128], FP32, name="x1b", tag="x1b")
                nc.sync.dma_start(out=x1b[:], in_=x1T[:, blk_sl(b)])
                py0 = psyp.tile([128, 128], FP32, name="py0", tag="py", padded_shape=[128, 256])
                py1 = psyp.tile([128, 128], FP32, name="py1", tag="py", padded_shape=[128, 256])
                nc.tensor.matmul(py0[:], lhsT=x0b[:], rhs=wl0t[:], start=True, stop=True)
                nc.tensor.matmul(py1[:], lhsT=x1b[:], rhs=wl1t[:], start=True, stop=True)
                evy = evp.tile([128, 256], FP32, name="evy", tag="evy")
                nc.scalar.copy(out=evy[:, 0:128], in_=py0[:])
                nc.vector.tensor_copy(out=evy[:, 128:256], in_=py1[:])
                nc.sync.dma_start(out=y01_own[blk_sl(b), :], in_=evy[:])

            nc.gpsimd.collective_compute(
                "AllGather", AL.bypass, replica_groups=RG,
                ins=[y01_own[:]], outs=[Y01[:]])

            # ================= aggregation layer (L1/L2) =====================
            def agg_layer(Ytab, wr_tiles, bias_t, h_src, h_dst, wl_next, y_next,
                          d_next):
                tabA = Ytab[0:HALF, :]
                tabB = Ytab[HALF:NCORES * SP, :]
                gtiles = {}
                qn = [0]

                def emit_gathers(cis):
                    for ci in cis:
                        cl = calls[ci]
                        k = cl["k"]
                        g = gp.tile([128, TPC, D1], FP32, name="g", tag="g")
                        nc.gpsimd.dma_gather(
                            out_ap=g[:, 0:k, :],
                            in_ap=(tabA if cl["h"] == 0 else tabB),
                            idxs_ap=idxt[:, ci * 32: ci * 32 + (k * 128) // 16],
                            num_idxs=k * 128, num_idxs_reg=k * 128,
                            elem_size=D1, queue_num=qn[0] % 4)
                        qn[0] += 1
                        gtiles[ci] = g

                for b in range(NBLK):
                    cis = sorted({ci for ci, _, _ in block_tiles[b]})
                    emit_gathers(cis)
                    ps0 = psp.tile([128, 128], FP32, name="ps0", tag="ps0")
                    ps1 = psp.tile([128, 128], FP32, name="ps1", tag="ps1")
                    if h_src is None:
                        x0b = xsp.tile([128, 128], FP32, name="x0b2", tag="x0b")
                        nc.sync.dma_start(out=x0b[:], in_=x0T[:, blk_sl(b)])
                        x1b = xsp.tile([128, 128], FP32, name="x1b2", tag="x1b")
                        nc.sync.dma_start(out=x1b[:], in_=x1T[:, blk_sl(b)])
                        nc.tensor.matmul(ps0[:], lhsT=wr0t[:], rhs=x0b[:], start=True, stop=False)
                        nc.tensor.matmul(ps1[:], lhsT=wr1t[:], rhs=x1b[:], start=True, stop=False)
                    else:
                        hs = [h_src[0][:, blk_sl(b)], h_src[1][:, blk_sl(b)]]
                        nc.tensor.matmul(ps0[:], lhsT=wr_tiles[0][:, 0:128], rhs=hs[0], start=True, stop=False)
                        nc.tensor.matmul(ps0[:], lhsT=wr_tiles[1][:, 0:128], rhs=hs[1], start=False, stop=False)
                        nc.tensor.matmul(ps1[:], lhsT=wr_tiles[0][:, 128:256], rhs=hs[0], start=True, stop=False)
                        nc.tensor.matmul(ps1[:], lhsT=wr_tiles[1][:, 128:256], rhs=hs[1], start=False, stop=False)
                    nc.tensor.matmul(ps0[:], lhsT=bias_t[0:1, 0:128], rhs=ones_r[0:1, :],
                                     start=False, stop=False)
                    nc.tensor.matmul(ps1[:], lhsT=bias_t[0:1, 128:256], rhs=ones_r[0:1, :],
                                     start=False, stop=False)
                    tl = block_tiles[b]
                    for n, (ci, slot, dcol) in enumerate(tl):
                        g = gtiles[ci]
                        oh = onehot(dcol)
                        last = (n == len(tl) - 1)
                        nc.tensor.matmul(ps0[:], lhsT=g[:, slot, 0:128], rhs=oh[:],
                                         start=False, stop=last)
                        nc.tensor.matmul(ps1[:], lhsT=g[:, slot, 128:256], rhs=oh[:],
                                         start=False, stop=last)
                    nc.scalar.activation(h_dst[0][:, blk_sl(b)], ps0[:], AF.Relu)
                    nc.scalar.activation(h_dst[1][:, blk_sl(b)], ps1[:], AF.Relu)
                    pyn = psyp.tile([128, d_next], FP32, name="pyn", tag="py",
                                    padded_shape=[128, 256])
                    nc.tensor.matmul(pyn[:], lhsT=h_dst[0][:, blk_sl(b)], rhs=wl_next[0][:],
                                     start=True, stop=False)
                    nc.tensor.matmul(pyn[:], lhsT=h_dst[1][:, blk_sl(b)], rhs=wl_next[1][:],
                                     start=False, stop=True)
                    evn = evp.tile([128, d_next], FP32, name="evn", tag="evy",
                                   padded_shape=[128, 256])
                    nc.scalar.copy(out=evn[:], in_=pyn[:])
                    nc.sync.dma_start(out=y_next[blk_sl(b), :], in_=evn[:])

            agg_layer(Y01, None, b01t, None, hT, wlmt, ym_own, DM)
            nc.gpsimd.collective_compute(
                "AllGather", AL.bypass, replica_groups=RG,
                ins=[ym_own[:]], outs=[Ym[:]])
            agg_layer(Ym, wrmt, bmt, hT, h2T, wlot, yo_own, DO)
            nc.gpsimd.collective_compute(
                "AllGather", AL.bypass, replica_groups=RG,
                ins=[yo_own[:]], outs=[Yo[:]])

            # ================= L3: out[node, 64] ============================
            tabA = Yo[0:HALF, :]
            tabB = Yo[HALF:NCORES * SP, :]
            qn3 = 0
            for b in range(NBLK):
                gtiles3 = {}
                cis = sorted({ci for ci, _, _ in block_tiles[b]})
                for ci in cis:
                    cl = calls[ci]
                    k = cl["k"]
                    g3 = gp.tile([128, TPC, DO], FP32, name="g3", tag="g3")
                    nc.gpsimd.dma_gather(
                        out_ap=g3[:, 0:k, :], in_ap=(tabA if cl["h"] == 0 else tabB),
                        idxs_ap=idxt[:, ci * 32: ci * 32 + (k * 128) // 16],
                        num_idxs=k * 128, num_idxs_reg=k * 128,
                        elem_size=DO, queue_num=qn3 % 4)
                    qn3 += 1
                    gtiles3[ci] = g3
                ps3 = psp.tile([128, DO], FP32, name="ps3", tag="ps0",
                               padded_shape=[128, 128])
                nc.tensor.matmul(ps3[:], lhsT=h2T[0][:, blk_sl(b)], rhs=wrot[0][:],
                                 start=True, stop=False)
                nc.tensor.matmul(ps3[:], lhsT=h2T[1][:, blk_sl(b)], rhs=wrot[1][:],
                                 start=False, stop=False)
                nc.tensor.matmul(ps3[:], lhsT=ones_r[0:1, :], rhs=bot[0:1, :],
                                 start=False, stop=False)
                tl = block_tiles[b]
                for n, (ci, slot, dcol) in enumerate(tl):
                    g3 = gtiles3[ci]
                    oh = onehot(dcol)
                    nc.tensor.matmul(ps3[:], lhsT=oh[:], rhs=g3[:, slot, :],
                                     start=False, stop=(n == len(tl) - 1))
                osb = evp.tile([128, DO], FP32, name="osb", tag="osb")
                nc.scalar.copy(out=osb[:], in_=ps3[:])
                rows = min(128, S - b * 128)
                nc.sync.dma_start(out=outd[b * 128: b * 128 + rows, :],
                                  in_=osb[0:rows, :])

    nc.finalize()
    return nc


_CACHE = {}


def _make_inmaps(inputs, pre, calls):
    x0 = np.asarray(inputs["x0"], np.float32)
    x1 = np.asarray(inputs["x1"], np.float32)
    deg = pre["deg"]
    in_maps = []
    for c in range(NCORES):
        invd_local = (1.0 / np.maximum(deg[c], 1.0)).astype(np.float64)
        idx_img, dstloc, dstw = _idx_arrays(pre, calls, c, invd_local)
        x0c = np.zeros((128, SP), np.float32)
        x0c[:, :S] = x0[c * S:(c + 1) * S, :].T
        x1c = np.zeros((128, SP), np.float32)
        x1c[:, :S] = x1[c * S:(c + 1) * S, :].T
        in_maps.append({
            "x0T": x0c, "x1T": x1c,
            "wl0": np.asarray(inputs["Wl0"], np.float32),
            "wr0": np.asarray(inputs["Wr0"], np.float32),
            "wl1": np.asarray(inputs["Wl1"], np.float32),
            "wr1": np.asarray(inputs["Wr1"], np.float32),
            "wlm": np.asarray(inputs["Wlm"], np.float32),
            "wrm": np.asarray(inputs["Wrm"], np.float32),
            "wlo": np.asarray(inputs["Wlo"], np.float32),
            "wro": np.asarray(inputs["Wro"], np.float32),
            "b01": np.concatenate([np.asarray(inputs["b0"], np.float32),
                                   np.asarray(inputs["b1"], np.float32)])[None, :],
            "bm": np.asarray(inputs["bm"], np.float32)[None, :],
            "bo": np.asarray(inputs["bo"], np.float32)[None, :],
            "idx": idx_img, "dstl": dstloc, "dstw": dstw,
        })
    return in_maps


def _get_program(edge_index):
    if "prog" in _CACHE:
        return _CACHE["prog"]
    pre = _preprocess(edge_index)
    calls, block_tiles, ht_tiles = _build_callplan(pre["tiles_hb"])
    tiles_total = int(pre["tiles_hb"].sum())
    nc = _build_bass(pre, calls, block_tiles, ht_tiles, len(calls) * 32, tiles_total)
    _CACHE["prog"] = (nc, pre, calls)
    return _CACHE["prog"]


LAST_EXEC_NS = None


def kernel(**inputs):
    global LAST_EXEC_NS
    _install_hooks()
    from concourse.bass_utils import run_bass_kernel_spmd

    nc, pre, calls = _get_program(inputs["edge_index"])
    in_maps = _make_inmaps(inputs, pre, calls)
    trace = os.environ.get("KERNEL_TRACE", "0") == "1"
    res = run_bass_kernel_spmd(nc, in_maps, list(range(NCORES)), trace=trace)
    LAST_EXEC_NS = res.exec_time_ns
    return np.concatenate([np.asarray(res.results[c]["out"]) for c in range(NCORES)], axis=0)


# revision 8
# speedup vs baseline: 1.3871x; 1.3871x over previous
"""Self-contained Trainium2 Bass kernel for 4-layer GraphSAGE (nn_LASAGE).

Strategy:
  - Nodes dst-sharded across 8 cores (6250/core, padded to 6272 = 49 blocks of 128).
  - Aggregation is done POST-matmul: agg(x)@Wl == agg(x@Wl), so per layer each
    core computes y = h @ Wl for its own shard, an AllGather replicates the full
    Y table [50176, d] to every core's DRAM, and edges gather y[src] rows with
    dma_gather (int16 idx -> table split in two halves, 4 SWDGE queues).
  - Scatter-add into dst blocks via weighted-one-hot matmuls on the PE:
    psumT[feat, dst] += G[e, feat] contracted with onehot[e, dst], where
    onehot[e, dst] = (iota == dstcol[e]) * invdeg[dst[e]] is built in a single
    DVE tensor_scalar op. The Wr-side and bias (K=1 matmul vs ones) accumulate
    into the same PSUM, so the epilogue is a single fused relu.
  - Layer1 fuses conv0+conv1 (concat -> 256 feat). Layer3 (output, d=64) uses
    non-transposed psum so rows DMA straight to the output.
"""
import sys, os, types

sys.path.insert(0, "/opt/trn_rl_repo")
import numpy as np

N = 50000
E = 800000
NCORES = 8
S = N // NCORES            # 6250 real nodes per core
SP = 6272                  # padded (49 blocks of 128)
NBLK = SP // 128
SPH = SP // 2              # 3136: local-row split for the two AG half-tables
HALF = NCORES * SPH        # 25088 rows per half-table (int16-safe)
D1 = 256                   # concat(h0, h1)
DM = 256
DO = 64
MAXI = 512                 # max idxs per dma_gather call
TPC = MAXI // 128          # tiles per full call


def _install_hooks():
    """antenv.axon_hooks shim so trace=True works in this image (optional)."""
    try:
        import antenv
        if "antenv.axon_hooks" not in sys.modules:
            mod = types.ModuleType("antenv.axon_hooks")
            mod._hook = None
            mod.set_axon_ntff_profile_hook = lambda h: setattr(mod, "_hook", h)
            mod.get_axon_ntff_profile_hook = lambda: mod._hook
            sys.modules["antenv.axon_hooks"] = mod
            antenv.axon_hooks = mod
        from antenv.axon_hooks import get_axon_ntff_profile_hook, set_axon_ntff_profile_hook
        if get_axon_ntff_profile_hook() is None:
            from trn_agent_boot.trn_boot import _ntff_profile_via_ctypes
            set_axon_ntff_profile_hook(_ntff_profile_via_ctypes("/opt/axon/libaxon_pjrt.so"))
        import concourse.bass_utils as bu
        bu.upload_artifacts = lambda tmpdir: f"file://{tmpdir}"
    except Exception:
        pass


def _preprocess(edge_index):
    """Edge lists per core, grouped by (dst block, src half), padded per-tile."""
    src = np.asarray(edge_index[0], np.int64)
    dst = np.asarray(edge_index[1], np.int64)
    core = dst // S
    dl = (dst % S).astype(np.int64)
    blk = dl // 128
    col = dl % 128
    sloc = src % S
    half = (sloc >= SPH).astype(np.int64)
    grow = (src // S) * SPH + (sloc - half * SPH)   # row within its half-table

    deg = np.bincount(core * S + dl, minlength=N).reshape(NCORES, S)

    order = np.lexsort((grow, blk, half, core))
    core_s, half_s, blk_s, col_s, row_s = (core[order], half[order], blk[order],
                                           col[order], grow[order])

    key = (core_s * 2 + half_s) * NBLK + blk_s
    counts = np.bincount(key, minlength=NCORES * 2 * NBLK).reshape(NCORES, 2, NBLK)
    tiles_hb = np.ceil(counts.max(axis=0) / 128).astype(np.int64)   # [2, NBLK]
    tiles_hb = np.maximum(tiles_hb, 1)

    pad_hb = tiles_hb * 128
    tot_h = pad_hb.sum(axis=1)
    seg_off = np.zeros((2, NBLK), np.int64)
    seg_off[:, 1:] = np.cumsum(pad_hb, axis=1)[:, :-1]

    srcpad = np.zeros((NCORES, 2), dtype=object)
    colpad = np.zeros((NCORES, 2), dtype=object)
    for c in range(NCORES):
        for h in range(2):
            srcpad[c, h] = np.zeros(int(tot_h[h]), np.int64)
            colpad[c, h] = np.full(int(tot_h[h]), -1, np.int64)
    grp = key
    first = np.r_[True, grp[1:] != grp[:-1]]
    gidx = np.arange(len(grp)) - np.maximum.accumulate(np.where(first, np.arange(len(grp)), 0))
    pos = seg_off[half_s, blk_s] + gidx
    for c in range(NCORES):
        m = core_s == c
        for h in range(2):
            mh = m & (half_s == h)
            p = pos[mh]
            srcpad[c, h][p] = row_s[mh]
            colpad[c, h][p] = col_s[mh]

    return {
        "tiles_hb": tiles_hb, "seg_off": seg_off,
        "srcpad": srcpad, "colpad": colpad, "deg": deg,
    }


def _build_callplan(tiles_hb):
    """Gather call plan (compile-time, same for every core)."""
    calls = []
    block_tiles = {b: [] for b in range(NBLK)}
    tile_ctr = [0, 0]
    ht_tiles = [int(tiles_hb[0].sum()), int(tiles_hb[1].sum())]
    for b in range(NBLK):
        for h in range(2):
            nt = int(tiles_hb[h, b])
            done = 0
            while done < nt:
                k = min(TPC, nt - done)
                ci = len(calls)
                dcol0 = (0 if h == 0 else ht_tiles[0]) + tile_ctr[h]
                calls.append(dict(h=h, k=k, tile_base=tile_ctr[h], blk=b, dcol0=dcol0))
                for j in range(k):
                    dcol = (0 if h == 0 else ht_tiles[0]) + tile_ctr[h] + j
                    block_tiles[b].append((ci, j, dcol))
                tile_ctr[h] += k
                done += k
    return calls, block_tiles, ht_tiles


def _idx_arrays(pre, calls, core, invd_local):
    """int16 idx image [128, ncalls*32], dstloc & dstw [128, ntiles] f32."""
    ncalls = len(calls)
    idx_img = np.zeros((16, ncalls * 32), np.int16)
    tiles_total = int(pre["tiles_hb"].sum())
    dstloc = np.full((128, tiles_total), -1.0, np.float32)
    dstw = np.zeros((128, tiles_total), np.float32)
    ht0 = int(pre["tiles_hb"][0].sum())
    for ci, cl in enumerate(calls):
        h, k, tb, b = cl["h"], cl["k"], cl["tile_base"], cl["blk"]
        e0 = int(pre["seg_off"][h, b]) + (tb - int(pre["tiles_hb"][h, :b].sum())) * 128
        nidx = k * 128
        seg_src = pre["srcpad"][core, h][e0:e0 + nidx]
        seg_col = pre["colpad"][core, h][e0:e0 + nidx]
        idx_img[:, ci * 32: ci * 32 + (nidx // 16)] = seg_src.reshape(-1, 16).T.astype(np.int16)
        for t in range(k):
            dcol = (0 if h == 0 else ht0) + tb + t
            cc = seg_col[t * 128:(t + 1) * 128]
            dstloc[:, dcol] = cc
            w = np.where(cc >= 0, invd_local[np.clip(b * 128 + cc, 0, S - 1)], 0.0)
            dstw[:, dcol] = w.astype(np.float32)
    return np.tile(idx_img, (8, 1)), dstloc, dstw


def _build_bass(pre, calls, block_tiles, ht_tiles, ncalls_cols, tiles_total):
    import concourse.bass as bass
    import concourse.bacc as bacc
    import concourse.mybir as mybir
    import concourse.tile as tile

    FP32 = mybir.dt.float32
    I16 = mybir.dt.int16
    AL = mybir.AluOpType
    AF = mybir.ActivationFunctionType

    nc = bacc.Bacc("TRN2", target_bir_lowering=False, debug=False,
                   enable_asserts=False, num_devices=NCORES, num_swdge_queues=4)

    x0T = nc.dram_tensor("x0T", [128, SP], FP32, kind="ExternalInput")
    x1T = nc.dram_tensor("x1T", [128, SP], FP32, kind="ExternalInput")
    wl0 = nc.dram_tensor("wl0", [128, 128], FP32, kind="ExternalInput")
    wr0 = nc.dram_tensor("wr0", [128, 128], FP32, kind="ExternalInput")
    wl1 = nc.dram_tensor("wl1", [128, 128], FP32, kind="ExternalInput")
    wr1 = nc.dram_tensor("wr1", [128, 128], FP32, kind="ExternalInput")
    wlm = nc.dram_tensor("wlm", [256, 256], FP32, kind="ExternalInput")
    wrm = nc.dram_tensor("wrm", [256, 256], FP32, kind="ExternalInput")
    wlo = nc.dram_tensor("wlo", [256, 64], FP32, kind="ExternalInput")
    wro = nc.dram_tensor("wro", [256, 64], FP32, kind="ExternalInput")
    b01d = nc.dram_tensor("b01", [1, 256], FP32, kind="ExternalInput")
    bmd = nc.dram_tensor("bm", [1, 256], FP32, kind="ExternalInput")
    bod = nc.dram_tensor("bo", [1, 64], FP32, kind="ExternalInput")
    idxd = nc.dram_tensor("idx", [128, ncalls_cols], I16, kind="ExternalInput")
    dstld = nc.dram_tensor("dstl", [128, tiles_total], FP32, kind="ExternalInput")
    dstwd = nc.dram_tensor("dstw", [128, tiles_total], FP32, kind="ExternalInput")
    outd = nc.dram_tensor("out", [S, DO], FP32, kind="ExternalOutput")

    with tile.TileContext(nc) as tc:
        with (
            tc.tile_pool(name="const", bufs=1) as cp,
            tc.tile_pool(name="acts", bufs=1) as hp,
            tc.tile_pool(name="g", bufs=6) as gp,
            tc.tile_pool(name="oh", bufs=4) as ohp,
            tc.tile_pool(name="xs", bufs=2) as xsp,
            # PSUM budget (8 banks): ps0(2) + ps1(2) + py(2) = 6 banks
            tc.tile_pool(name="ps", bufs=2, space="PSUM") as psp,
            tc.tile_pool(name="psy", bufs=2, space="PSUM") as psyp,
            tc.tile_pool(name="ev", bufs=2) as evp,
            tc.tile_pool(name="dram", bufs=1, space="DRAM") as dp,
        ):
            def load(name, dt_, shape, src):
                t = cp.tile(shape, dt_, name=name)
                nc.sync.dma_start(out=t[:], in_=src)
                return t

            wl0t = load("wl0t", FP32, [128, 128], wl0[:])
            wr0t = load("wr0t", FP32, [128, 128], wr0[:])
            wl1t = load("wl1t", FP32, [128, 128], wl1[:])
            wr1t = load("wr1t", FP32, [128, 128], wr1[:])
            wlmt = [load(f"wlmt{i}", FP32, [128, 256], wlm[i * 128:(i + 1) * 128, :]) for i in range(2)]
            wrmt = [load(f"wrmt{i}", FP32, [128, 256], wrm[i * 128:(i + 1) * 128, :]) for i in range(2)]
            wlot = [load(f"wlot{i}", FP32, [128, 64], wlo[i * 128:(i + 1) * 128, :]) for i in range(2)]
            wrot = [load(f"wrot{i}", FP32, [128, 64], wro[i * 128:(i + 1) * 128, :]) for i in range(2)]
            b01t = load("b01t", FP32, [1, 256], b01d[:])
            bmt = load("bmt", FP32, [1, 256], bmd[:])
            bot = load("bot", FP32, [1, 64], bod[:])
            idxt = load("idxt", I16, [128, ncalls_cols], idxd[:])
            dstl = load("dstlt", FP32, [128, tiles_total], dstld[:])
            dstw = load("dstwt", FP32, [128, tiles_total], dstwd[:])

            ones_r = cp.tile([1, 128], FP32, name="ones_r")
            nc.vector.memset(ones_r[:], 1.0)
            iota_i = cp.tile([128, TPC, 128], mybir.dt.int32, name="iota_i")
            nc.gpsimd.iota(iota_i[:], pattern=[[0, TPC], [1, 128]], base=0,
                           channel_multiplier=0)
            iota_f = cp.tile([128, TPC, 128], FP32, name="iota_f")
            nc.vector.tensor_copy(out=iota_f[:], in_=iota_i[:])

            hT = [hp.tile([128, SP], FP32, name=f"hT{i}") for i in range(2)]
            h2T = [hp.tile([128, SP], FP32, name=f"h2T{i}") for i in range(2)]

            shared = "Shared" if NCORES > 4 else "Local"
            y01_own = [dp.tile([SPH, D1], FP32, name=f"y01_own{h}") for h in range(2)]
            Y01 = [dp.tile([HALF, D1], FP32, name=f"Y01{h}", addr_space=shared) for h in range(2)]
            ym_own = [dp.tile([SPH, DM], FP32, name=f"ym_own{h}") for h in range(2)]
            Ym = [dp.tile([HALF, DM], FP32, name=f"Ym{h}", addr_space=shared) for h in range(2)]
            yo_own = [dp.tile([SPH, DO], FP32, name=f"yo_own{h}") for h in range(2)]
            Yo = [dp.tile([HALF, DO], FP32, name=f"Yo{h}", addr_space=shared) for h in range(2)]

            def write_y(dsts, b, src_tile, d):
                r0 = b * 128
                if r0 + 128 <= SPH:
                    nc.sync.dma_start(out=dsts[0][r0:r0 + 128, :], in_=src_tile[:])
                elif r0 >= SPH:
                    nc.sync.dma_start(out=dsts[1][r0 - SPH:r0 - SPH + 128, :], in_=src_tile[:])
                else:
                    nlo = SPH - r0
                    nc.sync.dma_start(out=dsts[0][r0:SPH, :], in_=src_tile[0:nlo, :])
                    nc.sync.dma_start(out=dsts[1][0:128 - nlo, :], in_=src_tile[nlo:128, :])

            RG = [list(range(NCORES))]

            def blk_sl(b):
                return slice(b * 128, (b + 1) * 128)

            def onehot_call(cl):
                k, d0 = cl["k"], cl["dcol0"]
                oh = ohp.tile([128, TPC, 128], FP32, name="oh", tag="oh")
                nc.vector.tensor_tensor(
                    out=oh[:, 0:k, :], in0=iota_f[:, 0:k, :],
                    in1=dstl[:, d0:d0 + k].to_broadcast([128, k, 128]),
                    op=AL.is_equal)
                nc.vector.tensor_tensor(
                    out=oh[:, 0:k, :], in0=oh[:, 0:k, :],
                    in1=dstw[:, d0:d0 + k].to_broadcast([128, k, 128]),
                    op=AL.mult)
                return oh

            # ================= L1 pre: y01_own = [x0@Wl0 | x1@Wl1] =========
            for b in range(NBLK):
                x0b = xsp.tile([128, 128], FP32, name="x0b", tag="x0b")
                nc.sync.dma_start(out=x0b[:], in_=x0T[:, blk_sl(b)])
                x1b = xsp.tile([128, 128], FP32, name="x1b", tag="x1b")
                nc.sync.dma_start(out=x1b[:], in_=x1T[:, blk_sl(b)])
                py0 = psyp.tile([128, 128], FP32, name="py0", tag="py", padded_shape=[128, 256])
                py1 = psyp.tile([128, 128], FP32, name="py1", tag="py", padded_shape=[128, 256])
                nc.tensor.matmul(py0[:], lhsT=x0b[:], rhs=wl0t[:], start=True, stop=True)
                nc.tensor.matmul(py1[:], lhsT=x1b[:], rhs=wl1t[:], start=True, stop=True)
                evy = evp.tile([128, 256], FP32, name="evy", tag="evy")
                nc.vector.tensor_copy(out=evy[:, 0:128], in_=py0[:])
                nc.vector.tensor_copy(out=evy[:, 128:256], in_=py1[:])
                write_y(y01_own, b, evy, D1)

            for h in range(2):
                nc.gpsimd.collective_compute(
                    "AllGather", AL.bypass, replica_groups=RG,
                    ins=[y01_own[h][:]], outs=[Y01[h][:]])

            # ================= aggregation layer (L1/L2) =====================
            def agg_layer(Ytab, wr_tiles, bias_t, h_src, h_dst, wl_next, y_next,
                          d_next):
                gtiles = {}
                ohs = {}
                qn = [0]

                def emit_gathers(cis):
                    for ci in cis:
                        cl = calls[ci]
                        k = cl["k"]
                        g = gp.tile([128, TPC, D1], FP32, name="g", tag="g")
                        nc.gpsimd.dma_gather(
                            out_ap=g[:, 0:k, :],
                            in_ap=Ytab[cl["h"]][:],
                            idxs_ap=idxt[:, ci * 32: ci * 32 + (k * 128) // 16],
                            num_idxs=k * 128, num_idxs_reg=k * 128,
                            elem_size=D1, queue_num=qn[0] % 4)
                        qn[0] += 1
                        gtiles[ci] = g
                        ohs[ci] = onehot_call(cl)

                for b in range(NBLK):
                    cis = sorted({ci for ci, _, _ in block_tiles[b]})
                    emit_gathers(cis)
                    ps0 = psp.tile([128, 128], FP32, name="ps0", tag="ps0")
                    ps1 = psp.tile([128, 128], FP32, name="ps1", tag="ps1")
                    if h_src is None:
                        x0b = xsp.tile([128, 128], FP32, name="x0b2", tag="x0b")
                        nc.sync.dma_start(out=x0b[:], in_=x0T[:, blk_sl(b)])
                        x1b = xsp.tile([128, 128], FP32, name="x1b2", tag="x1b")
                        nc.sync.dma_start(out=x1b[:], in_=x1T[:, blk_sl(b)])
                        nc.tensor.matmul(ps0[:], lhsT=wr0t[:], rhs=x0b[:], start=True, stop=False)
                        nc.tensor.matmul(ps1[:], lhsT=wr1t[:], rhs=x1b[:], start=True, stop=False)
                    else:
                        hs = [h_src[0][:, blk_sl(b)], h_src[1][:, blk_sl(b)]]
                        nc.tensor.matmul(ps0[:], lhsT=wr_tiles[0][:, 0:128], rhs=hs[0], start=True, stop=False)
                        nc.tensor.matmul(ps0[:], lhsT=wr_tiles[1][:, 0:128], rhs=hs[1], start=False, stop=False)
                        nc.tensor.matmul(ps1[:], lhsT=wr_tiles[0][:, 128:256], rhs=hs[0], start=True, stop=False)
                        nc.tensor.matmul(ps1[:], lhsT=wr_tiles[1][:, 128:256], rhs=hs[1], start=False, stop=False)
                    nc.tensor.matmul(ps0[:], lhsT=bias_t[0:1, 0:128], rhs=ones_r[0:1, :],
                                     start=False, stop=False)
                    nc.tensor.matmul(ps1[:], lhsT=bias_t[0:1, 128:256], rhs=ones_r[0:1, :],
                                     start=False, stop=False)
                    tl = block_tiles[b]
                    for n, (ci, slot, dcol) in enumerate(tl):
                        g = gtiles[ci]
                        oh = ohs[ci]
                        last = (n == len(tl) - 1)
                        nc.tensor.matmul(ps0[:], lhsT=g[:, slot, 0:128], rhs=oh[:, slot, :],
                                         start=False, stop=last)
                        nc.tensor.matmul(ps1[:], lhsT=g[:, slot, 128:256], rhs=oh[:, slot, :],
                                         start=False, stop=last)
                    nc.scalar.activation(h_dst[0][:, blk_sl(b)], ps0[:], AF.Relu)
                    nc.scalar.activation(h_dst[1][:, blk_sl(b)], ps1[:], AF.Relu)
                    pyn = psyp.tile([128, d_next], FP32, name="pyn", tag="py",
                                    padded_shape=[128, 256])
                    nc.tensor.matmul(pyn[:], lhsT=h_dst[0][:, blk_sl(b)], rhs=wl_next[0][:],
                                     start=True, stop=False)
                    nc.tensor.matmul(pyn[:], lhsT=h_dst[1][:, blk_sl(b)], rhs=wl_next[1][:],
                                     start=False, stop=True)
                    evn = evp.tile([128, d_next], FP32, name="evn", tag="evy",
                                   padded_shape=[128, 256])
                    nc.vector.tensor_copy(out=evn[:], in_=pyn[:])
                    write_y(y_next, b, evn, d_next)

            agg_layer(Y01, None, b01t, None, hT, wlmt, ym_own, DM)
            for h in range(2):
                nc.gpsimd.collective_compute(
                    "AllGather", AL.bypass, replica_groups=RG,
                    ins=[ym_own[h][:]], outs=[Ym[h][:]])
            agg_layer(Ym, wrmt, bmt, hT, h2T, wlot, yo_own, DO)
            for h in range(2):
                nc.gpsimd.collective_compute(
                    "AllGather", AL.bypass, replica_groups=RG,
                    ins=[yo_own[h][:]], outs=[Yo[h][:]])

            # ================= L3: out[node, 64] ============================
            qn3 = 0
            for b in range(NBLK):
                gtiles3 = {}
                ohs3 = {}
                cis = sorted({ci for ci, _, _ in block_tiles[b]})
                for ci in cis:
                    cl = calls[ci]
                    k = cl["k"]
                    g3 = gp.tile([128, TPC, DO], FP32, name="g3", tag="g3")
                    nc.gpsimd.dma_gather(
                        out_ap=g3[:, 0:k, :], in_ap=Yo[cl["h"]][:],
                        idxs_ap=idxt[:, ci * 32: ci * 32 + (k * 128) // 16],
                        num_idxs=k * 128, num_idxs_reg=k * 128,
                        elem_size=DO, queue_num=qn3 % 4)
                    qn3 += 1
                    gtiles3[ci] = g3
                    ohs3[ci] = onehot_call(cl)
                ps3 = psp.tile([128, DO], FP32, name="ps3", tag="ps0",
                               padded_shape=[128, 128])
                nc.tensor.matmul(ps3[:], lhsT=h2T[0][:, blk_sl(b)], rhs=wrot[0][:],
                                 start=True, stop=False)
                nc.tensor.matmul(ps3[:], lhsT=h2T[1][:, blk_sl(b)], rhs=wrot[1][:],
                                 start=False, stop=False)
                nc.tensor.matmul(ps3[:], lhsT=ones_r[0:1, :], rhs=bot[0:1, :],
                                 start=False, stop=False)
                tl = block_tiles[b]
                for n, (ci, slot, dcol) in enumerate(tl):
                    g3 = gtiles3[ci]
                    oh = ohs3[ci]
                    nc.tensor.matmul(ps3[:], lhsT=oh[:, slot, :], rhs=g3[:, slot, :],
                                     start=False, stop=(n == len(tl) - 1))
                osb = evp.tile([128, DO], FP32, name="osb", tag="osb")
                nc.vector.tensor_copy(out=osb[:], in_=ps3[:])
                rows = min(128, S - b * 128)
                nc.sync.dma_start(out=outd[b * 128: b * 128 + rows, :],
                                  in_=osb[0:rows, :])

    nc.finalize()
    return nc


_CACHE = {}


def _make_inmaps(inputs, pre, calls):
    x0 = np.asarray(inputs["x0"], np.float32)
    x1 = np.asarray(inputs["x1"], np.float32)
    deg = pre["deg"]
    in_maps = []
    for c in range(NCORES):
        invd_local = (1.0 / np.maximum(deg[c], 1.0)).astype(np.float64)
        idx_img, dstloc, dstw = _idx_arrays(pre, calls, c, invd_local)
        x0c = np.zeros((128, SP), np.float32)
        x0c[:, :S] = x0[c * S:(c + 1) * S, :].T
        x1c = np.zeros((128, SP), np.float32)
        x1c[:, :S] = x1[c * S:(c + 1) * S, :].T
        in_maps.append({
            "x0T": x0c, "x1T": x1c,
            "wl0": np.asarray(inputs["Wl0"], np.float32),
            "wr0": np.asarray(inputs["Wr0"], np.float32),
            "wl1": np.asarray(inputs["Wl1"], np.float32),
            "wr1": np.asarray(inputs["Wr1"], np.float32),
            "wlm": np.asarray(inputs["Wlm"], np.float32),
            "wrm": np.asarray(inputs["Wrm"], np.float32),
            "wlo": np.asarray(inputs["Wlo"], np.float32),
            "wro": np.asarray(inputs["Wro"], np.float32),
            "b01": np.concatenate([np.asarray(inputs["b0"], np.float32),
                                   np.asarray(inputs["b1"], np.float32)])[None, :],
            "bm": np.asarray(inputs["bm"], np.float32)[None, :],
            "bo": np.asarray(inputs["bo"], np.float32)[None, :],
            "idx": idx_img, "dstl": dstloc, "dstw": dstw,
        })
    return in_maps


def _get_program(edge_index):
    if "prog" in _CACHE:
        return _CACHE["prog"]
    pre = _preprocess(edge_index)
    calls, block_tiles, ht_tiles = _build_callplan(pre["tiles_hb"])
    tiles_total = int(pre["tiles_hb"].sum())
    nc = _build_bass(pre, calls, block_tiles, ht_tiles, len(calls) * 32, tiles_total)
    _CACHE["prog"] = (nc, pre, calls)
    return _CACHE["prog"]


LAST_EXEC_NS = None


def kernel(**inputs):
    global LAST_EXEC_NS
    _install_hooks()
    from concourse.bass_utils import run_bass_kernel_spmd

    nc, pre, calls = _get_program(inputs["edge_index"])
    in_maps = _make_inmaps(inputs, pre, calls)
    trace = os.environ.get("KERNEL_TRACE", "0") == "1"
    res = run_bass_kernel_spmd(nc, in_maps, list(range(NCORES)), trace=trace)
    LAST_EXEC_NS = res.exec_time_ns
    return np.concatenate([np.asarray(res.results[c]["out"]) for c in range(NCORES)], axis=0)


# revision 9
# speedup vs baseline: 1.7891x; 1.2898x over previous
"""Self-contained Trainium2 Bass kernel for 4-layer GraphSAGE (nn_LASAGE).

Strategy:
  - Nodes dst-sharded across 8 cores (6250/core, padded to 6272 = 49 blocks of 128).
  - Aggregation is done POST-matmul: agg(x)@Wl == agg(x@Wl), so per layer each
    core computes y = h @ Wl for its own shard, an AllGather replicates the full
    Y table [50176, d] to every core's DRAM, and edges gather y[src] rows with
    dma_gather (int16 idx -> table split in two halves, 4 SWDGE queues).
  - Scatter-add into dst blocks via weighted-one-hot matmuls on the PE:
    psumT[feat, dst] += G[e, feat] contracted with onehot[e, dst], where
    onehot[e, dst] = (iota == dstcol[e]) * invdeg[dst[e]] is built in a single
    DVE tensor_scalar op. The Wr-side and bias (K=1 matmul vs ones) accumulate
    into the same PSUM, so the epilogue is a single fused relu.
  - Layer1 fuses conv0+conv1 (concat -> 256 feat). Layer3 (output, d=64) uses
    non-transposed psum so rows DMA straight to the output.
"""
import sys, os, types

sys.path.insert(0, "/opt/trn_rl_repo")
import numpy as np

N = 50000
E = 800000
NCORES = 8
S = N // NCORES            # 6250 real nodes per core
SP = 6272                  # padded (49 blocks of 128)
NBLK = SP // 128
SPH = SP // 2              # 3136: local-row split for the two AG half-tables
HALF = NCORES * SPH        # 25088 rows per half-table (int16-safe)
D1 = 256                   # concat(h0, h1)
DM = 256
DO = 64
MAXI = 512                 # max idxs per dma_gather call
TPC = MAXI // 128          # tiles per full call
AGG_BF16 = os.environ.get("KERNEL_F32", "0") != "1"   # bf16 gather tables/one-hots


def _install_hooks():
    """antenv.axon_hooks shim so trace=True works in this image (optional)."""
    try:
        import antenv
        if "antenv.axon_hooks" not in sys.modules:
            mod = types.ModuleType("antenv.axon_hooks")
            mod._hook = None
            mod.set_axon_ntff_profile_hook = lambda h: setattr(mod, "_hook", h)
            mod.get_axon_ntff_profile_hook = lambda: mod._hook
            sys.modules["antenv.axon_hooks"] = mod
            antenv.axon_hooks = mod
        from antenv.axon_hooks import get_axon_ntff_profile_hook, set_axon_ntff_profile_hook
        if get_axon_ntff_profile_hook() is None:
            from trn_agent_boot.trn_boot import _ntff_profile_via_ctypes
            set_axon_ntff_profile_hook(_ntff_profile_via_ctypes("/opt/axon/libaxon_pjrt.so"))
        import concourse.bass_utils as bu
        bu.upload_artifacts = lambda tmpdir: f"file://{tmpdir}"
    except Exception:
        pass


def _preprocess(edge_index):
    """Edge lists per core, grouped by (dst block, src half), padded per-tile."""
    src = np.asarray(edge_index[0], np.int64)
    dst = np.asarray(edge_index[1], np.int64)
    core = dst // S
    dl = (dst % S).astype(np.int64)
    blk = dl // 128
    col = dl % 128
    sloc = src % S
    half = (sloc >= SPH).astype(np.int64)
    grow = (src // S) * SPH + (sloc - half * SPH)   # row within its half-table

    deg = np.bincount(core * S + dl, minlength=N).reshape(NCORES, S)

    order = np.lexsort((grow, blk, half, core))
    core_s, half_s, blk_s, col_s, row_s = (core[order], half[order], blk[order],
                                           col[order], grow[order])

    key = (core_s * 2 + half_s) * NBLK + blk_s
    counts = np.bincount(key, minlength=NCORES * 2 * NBLK).reshape(NCORES, 2, NBLK)
    tiles_hb = np.ceil(counts.max(axis=0) / 128).astype(np.int64)   # [2, NBLK]
    tiles_hb = np.maximum(tiles_hb, 1)

    pad_hb = tiles_hb * 128
    tot_h = pad_hb.sum(axis=1)
    seg_off = np.zeros((2, NBLK), np.int64)
    seg_off[:, 1:] = np.cumsum(pad_hb, axis=1)[:, :-1]

    srcpad = np.zeros((NCORES, 2), dtype=object)
    colpad = np.zeros((NCORES, 2), dtype=object)
    for c in range(NCORES):
        for h in range(2):
            srcpad[c, h] = np.zeros(int(tot_h[h]), np.int64)
            colpad[c, h] = np.full(int(tot_h[h]), -1, np.int64)
    grp = key
    first = np.r_[True, grp[1:] != grp[:-1]]
    gidx = np.arange(len(grp)) - np.maximum.accumulate(np.where(first, np.arange(len(grp)), 0))
    pos = seg_off[half_s, blk_s] + gidx
    for c in range(NCORES):
        m = core_s == c
        for h in range(2):
            mh = m & (half_s == h)
            p = pos[mh]
            srcpad[c, h][p] = row_s[mh]
            colpad[c, h][p] = col_s[mh]

    return {
        "tiles_hb": tiles_hb, "seg_off": seg_off,
        "srcpad": srcpad, "colpad": colpad, "deg": deg,
    }


def _build_callplan(tiles_hb):
    """Gather call plan (compile-time, same for every core)."""
    calls = []
    block_tiles = {b: [] for b in range(NBLK)}
    tile_ctr = [0, 0]
    ht_tiles = [int(tiles_hb[0].sum()), int(tiles_hb[1].sum())]
    for b in range(NBLK):
        for h in range(2):
            nt = int(tiles_hb[h, b])
            done = 0
            while done < nt:
                k = min(TPC, nt - done)
                ci = len(calls)
                dcol0 = (0 if h == 0 else ht_tiles[0]) + tile_ctr[h]
                calls.append(dict(h=h, k=k, tile_base=tile_ctr[h], blk=b, dcol0=dcol0))
                for j in range(k):
                    dcol = (0 if h == 0 else ht_tiles[0]) + tile_ctr[h] + j
                    block_tiles[b].append((ci, j, dcol))
                tile_ctr[h] += k
                done += k
    return calls, block_tiles, ht_tiles


def _idx_arrays(pre, calls, core, invd_local):
    """int16 idx image [128, ncalls*32], dstloc & dstw [128, ntiles] f32."""
    ncalls = len(calls)
    idx_img = np.zeros((16, ncalls * 32), np.int16)
    tiles_total = int(pre["tiles_hb"].sum())
    dstloc = np.full((128, tiles_total), -1.0, np.float32)
    dstw = np.zeros((128, tiles_total), np.float32)
    ht0 = int(pre["tiles_hb"][0].sum())
    for ci, cl in enumerate(calls):
        h, k, tb, b = cl["h"], cl["k"], cl["tile_base"], cl["blk"]
        e0 = int(pre["seg_off"][h, b]) + (tb - int(pre["tiles_hb"][h, :b].sum())) * 128
        nidx = k * 128
        seg_src = pre["srcpad"][core, h][e0:e0 + nidx]
        seg_col = pre["colpad"][core, h][e0:e0 + nidx]
        idx_img[:, ci * 32: ci * 32 + (nidx // 16)] = seg_src.reshape(-1, 16).T.astype(np.int16)
        for t in range(k):
            dcol = (0 if h == 0 else ht0) + tb + t
            cc = seg_col[t * 128:(t + 1) * 128]
            dstloc[:, dcol] = cc
            w = np.where(cc >= 0, invd_local[np.clip(b * 128 + cc, 0, S - 1)], 0.0)
            dstw[:, dcol] = w.astype(np.float32)
    return np.tile(idx_img, (8, 1)), dstloc, dstw


def _build_bass(pre, calls, block_tiles, ht_tiles, ncalls_cols, tiles_total):
    import concourse.bass as bass
    import concourse.bacc as bacc
    import concourse.mybir as mybir
    import concourse.tile as tile

    FP32 = mybir.dt.float32
    BF16 = mybir.dt.bfloat16
    YDT = BF16 if AGG_BF16 else FP32
    I16 = mybir.dt.int16
    AL = mybir.AluOpType
    AF = mybir.ActivationFunctionType

    nc = bacc.Bacc("TRN2", target_bir_lowering=False, debug=False,
                   enable_asserts=False, num_devices=NCORES, num_swdge_queues=4)

    x0T = nc.dram_tensor("x0T", [128, SP], FP32, kind="ExternalInput")
    x1T = nc.dram_tensor("x1T", [128, SP], FP32, kind="ExternalInput")
    wl0 = nc.dram_tensor("wl0", [128, 128], FP32, kind="ExternalInput")
    wr0 = nc.dram_tensor("wr0", [128, 128], FP32, kind="ExternalInput")
    wl1 = nc.dram_tensor("wl1", [128, 128], FP32, kind="ExternalInput")
    wr1 = nc.dram_tensor("wr1", [128, 128], FP32, kind="ExternalInput")
    wlm = nc.dram_tensor("wlm", [256, 256], FP32, kind="ExternalInput")
    wrm = nc.dram_tensor("wrm", [256, 256], FP32, kind="ExternalInput")
    wlo = nc.dram_tensor("wlo", [256, 64], FP32, kind="ExternalInput")
    wro = nc.dram_tensor("wro", [256, 64], FP32, kind="ExternalInput")
    b01d = nc.dram_tensor("b01", [1, 256], FP32, kind="ExternalInput")
    bmd = nc.dram_tensor("bm", [1, 256], FP32, kind="ExternalInput")
    bod = nc.dram_tensor("bo", [1, 64], FP32, kind="ExternalInput")
    idxd = nc.dram_tensor("idx", [128, ncalls_cols], I16, kind="ExternalInput")
    dstld = nc.dram_tensor("dstl", [128, tiles_total], FP32, kind="ExternalInput")
    dstwd = nc.dram_tensor("dstw", [128, tiles_total], FP32, kind="ExternalInput")
    dstwbd = nc.dram_tensor("dstwb", [128, tiles_total], YDT, kind="ExternalInput")
    outd = nc.dram_tensor("out", [S, DO], FP32, kind="ExternalOutput")

    with tile.TileContext(nc) as tc:
        with (
            tc.tile_pool(name="const", bufs=1) as cp,
            tc.tile_pool(name="acts", bufs=1) as hp,
            tc.tile_pool(name="g", bufs=6) as gp,
            tc.tile_pool(name="oh", bufs=4) as ohp,
            tc.tile_pool(name="xs", bufs=2) as xsp,
            # PSUM budget (8 banks): ps0(2) + ps1(2) + py(2) = 6 banks
            tc.tile_pool(name="ps", bufs=2, space="PSUM") as psp,
            tc.tile_pool(name="psy", bufs=2, space="PSUM") as psyp,
            tc.tile_pool(name="ev", bufs=2) as evp,
            tc.tile_pool(name="dram", bufs=1, space="DRAM") as dp,
        ):
            def load(name, dt_, shape, src):
                t = cp.tile(shape, dt_, name=name)
                nc.sync.dma_start(out=t[:], in_=src)
                return t

            wl0t = load("wl0t", FP32, [128, 128], wl0[:])
            wr0t = load("wr0t", FP32, [128, 128], wr0[:])
            wl1t = load("wl1t", FP32, [128, 128], wl1[:])
            wr1t = load("wr1t", FP32, [128, 128], wr1[:])
            wlmt = [load(f"wlmt{i}", FP32, [128, 256], wlm[i * 128:(i + 1) * 128, :]) for i in range(2)]
            wrmt = [load(f"wrmt{i}", FP32, [128, 256], wrm[i * 128:(i + 1) * 128, :]) for i in range(2)]
            wlot = [load(f"wlot{i}", FP32, [128, 64], wlo[i * 128:(i + 1) * 128, :]) for i in range(2)]
            wrot = [load(f"wrot{i}", FP32, [128, 64], wro[i * 128:(i + 1) * 128, :]) for i in range(2)]
            b01t = load("b01t", FP32, [1, 256], b01d[:])
            bmt = load("bmt", FP32, [1, 256], bmd[:])
            bot = load("bot", FP32, [1, 64], bod[:])
            idxt = load("idxt", I16, [128, ncalls_cols], idxd[:])
            dstl = load("dstlt", FP32, [128, tiles_total], dstld[:])
            dstw = load("dstwt", FP32, [128, tiles_total], dstwd[:])
            dstwb = load("dstwbt", YDT, [128, tiles_total], dstwbd[:])

            ones_r = cp.tile([1, 128], FP32, name="ones_r")
            nc.vector.memset(ones_r[:], 1.0)
            iota_i = cp.tile([128, TPC, 128], mybir.dt.int32, name="iota_i")
            nc.gpsimd.iota(iota_i[:], pattern=[[0, TPC], [1, 128]], base=0,
                           channel_multiplier=0)
            iota_f = cp.tile([128, TPC, 128], FP32, name="iota_f")
            nc.vector.tensor_copy(out=iota_f[:], in_=iota_i[:])

            hT = [hp.tile([128, SP], FP32, name=f"hT{i}") for i in range(2)]
            h2T = [hp.tile([128, SP], FP32, name=f"h2T{i}") for i in range(2)]

            shared = "Shared" if NCORES > 4 else "Local"
            y01_own = [dp.tile([SPH, D1], YDT, name=f"y01_own{h}") for h in range(2)]
            Y01 = [dp.tile([HALF, D1], YDT, name=f"Y01{h}", addr_space=shared) for h in range(2)]
            ym_own = [dp.tile([SPH, DM], YDT, name=f"ym_own{h}") for h in range(2)]
            Ym = [dp.tile([HALF, DM], YDT, name=f"Ym{h}", addr_space=shared) for h in range(2)]
            yo_own = [dp.tile([SPH, DO], FP32, name=f"yo_own{h}") for h in range(2)]
            Yo = [dp.tile([HALF, DO], FP32, name=f"Yo{h}", addr_space=shared) for h in range(2)]

            def write_y(dsts, b, src_tile, d):
                r0 = b * 128
                if r0 + 128 <= SPH:
                    nc.sync.dma_start(out=dsts[0][r0:r0 + 128, :], in_=src_tile[:])
                elif r0 >= SPH:
                    nc.sync.dma_start(out=dsts[1][r0 - SPH:r0 - SPH + 128, :], in_=src_tile[:])
                else:
                    nlo = SPH - r0
                    nc.sync.dma_start(out=dsts[0][r0:SPH, :], in_=src_tile[0:nlo, :])
                    nc.sync.dma_start(out=dsts[1][0:128 - nlo, :], in_=src_tile[nlo:128, :])

            RG = [list(range(NCORES))]

            def blk_sl(b):
                return slice(b * 128, (b + 1) * 128)

            def onehot_call(cl, dt, wsrc):
                k, d0 = cl["k"], cl["dcol0"]
                oh = ohp.tile([128, TPC, 128], dt, name="oh",
                              tag="oh" if dt == YDT else "ohf",
                              padded_shape=[128, TPC, 128])
                nc.vector.tensor_tensor(
                    out=oh[:, 0:k, :], in0=iota_f[:, 0:k, :],
                    in1=dstl[:, d0:d0 + k].to_broadcast([128, k, 128]),
                    op=AL.is_equal)
                nc.vector.tensor_tensor(
                    out=oh[:, 0:k, :], in0=oh[:, 0:k, :],
                    in1=wsrc[:, d0:d0 + k].to_broadcast([128, k, 128]),
                    op=AL.mult)
                return oh

            # ================= L1 pre: y01_own = [x0@Wl0 | x1@Wl1] =========
            for b in range(NBLK):
                x0b = xsp.tile([128, 128], FP32, name="x0b", tag="x0b")
                nc.sync.dma_start(out=x0b[:], in_=x0T[:, blk_sl(b)])
                x1b = xsp.tile([128, 128], FP32, name="x1b", tag="x1b")
                nc.sync.dma_start(out=x1b[:], in_=x1T[:, blk_sl(b)])
                py0 = psyp.tile([128, 128], FP32, name="py0", tag="py", padded_shape=[128, 256])
                py1 = psyp.tile([128, 128], FP32, name="py1", tag="py", padded_shape=[128, 256])
                nc.tensor.matmul(py0[:], lhsT=x0b[:], rhs=wl0t[:], start=True, stop=True)
                nc.tensor.matmul(py1[:], lhsT=x1b[:], rhs=wl1t[:], start=True, stop=True)
                evy = evp.tile([128, 256], YDT, name="evy", tag="evy")
                nc.vector.tensor_copy(out=evy[:, 0:128], in_=py0[:])
                nc.vector.tensor_copy(out=evy[:, 128:256], in_=py1[:])
                write_y(y01_own, b, evy, D1)

            for h in range(2):
                nc.gpsimd.collective_compute(
                    "AllGather", AL.bypass, replica_groups=RG,
                    ins=[y01_own[h][:]], outs=[Y01[h][:]])

            # ================= aggregation layer (L1/L2) =====================
            def agg_layer(Ytab, wr_tiles, bias_t, h_src, h_dst, wl_next, y_next,
                          d_next, ynext_dt):
                gtiles = {}
                ohs = {}
                qn = [0]

                def emit_gathers(cis):
                    for ci in cis:
                        cl = calls[ci]
                        k = cl["k"]
                        g = gp.tile([128, TPC, D1], YDT, name="g", tag="g")
                        nc.gpsimd.dma_gather(
                            out_ap=g[:, 0:k, :],
                            in_ap=Ytab[cl["h"]][:],
                            idxs_ap=idxt[:, ci * 32: ci * 32 + (k * 128) // 16],
                            num_idxs=k * 128, num_idxs_reg=k * 128,
                            elem_size=D1, queue_num=qn[0] % 4)
                        qn[0] += 1
                        gtiles[ci] = g
                        ohs[ci] = onehot_call(cl, YDT, dstwb)

                for b in range(NBLK):
                    cis = sorted({ci for ci, _, _ in block_tiles[b]})
                    emit_gathers(cis)
                    ps0 = psp.tile([128, 128], FP32, name="ps0", tag="ps0")
                    ps1 = psp.tile([128, 128], FP32, name="ps1", tag="ps1")
                    if h_src is None:
                        x0b = xsp.tile([128, 128], FP32, name="x0b2", tag="x0b")
                        nc.sync.dma_start(out=x0b[:], in_=x0T[:, blk_sl(b)])
                        x1b = xsp.tile([128, 128], FP32, name="x1b2", tag="x1b")
                        nc.sync.dma_start(out=x1b[:], in_=x1T[:, blk_sl(b)])
                        nc.tensor.matmul(ps0[:], lhsT=wr0t[:], rhs=x0b[:], start=True, stop=False)
                        nc.tensor.matmul(ps1[:], lhsT=wr1t[:], rhs=x1b[:], start=True, stop=False)
                    else:
                        hs = [h_src[0][:, blk_sl(b)], h_src[1][:, blk_sl(b)]]
                        nc.tensor.matmul(ps0[:], lhsT=wr_tiles[0][:, 0:128], rhs=hs[0], start=True, stop=False)
                        nc.tensor.matmul(ps0[:], lhsT=wr_tiles[1][:, 0:128], rhs=hs[1], start=False, stop=False)
                        nc.tensor.matmul(ps1[:], lhsT=wr_tiles[0][:, 128:256], rhs=hs[0], start=True, stop=False)
                        nc.tensor.matmul(ps1[:], lhsT=wr_tiles[1][:, 128:256], rhs=hs[1], start=False, stop=False)
                    nc.tensor.matmul(ps0[:], lhsT=bias_t[0:1, 0:128], rhs=ones_r[0:1, :],
                                     start=False, stop=False)
                    nc.tensor.matmul(ps1[:], lhsT=bias_t[0:1, 128:256], rhs=ones_r[0:1, :],
                                     start=False, stop=False)
                    tl = block_tiles[b]
                    for n, (ci, slot, dcol) in enumerate(tl):
                        g = gtiles[ci]
                        oh = ohs[ci]
                        last = (n == len(tl) - 1)
                        nc.tensor.matmul(ps0[:], lhsT=g[:, slot, 0:128], rhs=oh[:, slot, :],
                                         start=False, stop=last)
                        nc.tensor.matmul(ps1[:], lhsT=g[:, slot, 128:256], rhs=oh[:, slot, :],
                                         start=False, stop=last)
                    nc.scalar.activation(h_dst[0][:, blk_sl(b)], ps0[:], AF.Relu)
                    nc.scalar.activation(h_dst[1][:, blk_sl(b)], ps1[:], AF.Relu)
                    pyn = psyp.tile([128, d_next], FP32, name="pyn", tag="py",
                                    padded_shape=[128, 256])
                    nc.tensor.matmul(pyn[:], lhsT=h_dst[0][:, blk_sl(b)], rhs=wl_next[0][:],
                                     start=True, stop=False)
                    nc.tensor.matmul(pyn[:], lhsT=h_dst[1][:, blk_sl(b)], rhs=wl_next[1][:],
                                     start=False, stop=True)
                    evn = evp.tile([128, d_next], ynext_dt, name="evn", tag="evy",
                                   padded_shape=[128, 256])
                    nc.vector.tensor_copy(out=evn[:], in_=pyn[:])
                    write_y(y_next, b, evn, d_next)

            agg_layer(Y01, None, b01t, None, hT, wlmt, ym_own, DM, YDT)
            for h in range(2):
                nc.gpsimd.collective_compute(
                    "AllGather", AL.bypass, replica_groups=RG,
                    ins=[ym_own[h][:]], outs=[Ym[h][:]])
            agg_layer(Ym, wrmt, bmt, hT, h2T, wlot, yo_own, DO, FP32)
            for h in range(2):
                nc.gpsimd.collective_compute(
                    "AllGather", AL.bypass, replica_groups=RG,
                    ins=[yo_own[h][:]], outs=[Yo[h][:]])

            # ================= L3: out[node, 64] ============================
            qn3 = 0
            for b in range(NBLK):
                gtiles3 = {}
                ohs3 = {}
                cis = sorted({ci for ci, _, _ in block_tiles[b]})
                for ci in cis:
                    cl = calls[ci]
                    k = cl["k"]
                    g3 = gp.tile([128, TPC, DO], FP32, name="g3", tag="g3")
                    nc.gpsimd.dma_gather(
                        out_ap=g3[:, 0:k, :], in_ap=Yo[cl["h"]][:],
                        idxs_ap=idxt[:, ci * 32: ci * 32 + (k * 128) // 16],
                        num_idxs=k * 128, num_idxs_reg=k * 128,
                        elem_size=DO, queue_num=qn3 % 4)
                    qn3 += 1
                    gtiles3[ci] = g3
                    ohs3[ci] = onehot_call(cl, FP32, dstw)
                ps3 = psp.tile([128, DO], FP32, name="ps3", tag="ps0",
                               padded_shape=[128, 128])
                nc.tensor.matmul(ps3[:], lhsT=h2T[0][:, blk_sl(b)], rhs=wrot[0][:],
                                 start=True, stop=False)
                nc.tensor.matmul(ps3[:], lhsT=h2T[1][:, blk_sl(b)], rhs=wrot[1][:],
                                 start=False, stop=False)
                nc.tensor.matmul(ps3[:], lhsT=ones_r[0:1, :], rhs=bot[0:1, :],
                                 start=False, stop=False)
                tl = block_tiles[b]
                for n, (ci, slot, dcol) in enumerate(tl):
                    g3 = gtiles3[ci]
                    oh = ohs3[ci]
                    nc.tensor.matmul(ps3[:], lhsT=oh[:, slot, :], rhs=g3[:, slot, :],
                                     start=False, stop=(n == len(tl) - 1))
                osb = evp.tile([128, DO], FP32, name="osb", tag="osb")
                nc.vector.tensor_copy(out=osb[:], in_=ps3[:])
                rows = min(128, S - b * 128)
                nc.sync.dma_start(out=outd[b * 128: b * 128 + rows, :],
                                  in_=osb[0:rows, :])

    nc.finalize()
    return nc


_CACHE = {}


def _make_inmaps(inputs, pre, calls):
    import ml_dtypes as _ml
    x0 = np.asarray(inputs["x0"], np.float32)
    x1 = np.asarray(inputs["x1"], np.float32)
    deg = pre["deg"]
    in_maps = []
    for c in range(NCORES):
        invd_local = (1.0 / np.maximum(deg[c], 1.0)).astype(np.float64)
        idx_img, dstloc, dstw = _idx_arrays(pre, calls, c, invd_local)
        x0c = np.zeros((128, SP), np.float32)
        x0c[:, :S] = x0[c * S:(c + 1) * S, :].T
        x1c = np.zeros((128, SP), np.float32)
        x1c[:, :S] = x1[c * S:(c + 1) * S, :].T
        in_maps.append({
            "x0T": x0c, "x1T": x1c,
            "wl0": np.asarray(inputs["Wl0"], np.float32),
            "wr0": np.asarray(inputs["Wr0"], np.float32),
            "wl1": np.asarray(inputs["Wl1"], np.float32),
            "wr1": np.asarray(inputs["Wr1"], np.float32),
            "wlm": np.asarray(inputs["Wlm"], np.float32),
            "wrm": np.asarray(inputs["Wrm"], np.float32),
            "wlo": np.asarray(inputs["Wlo"], np.float32),
            "wro": np.asarray(inputs["Wro"], np.float32),
            "b01": np.concatenate([np.asarray(inputs["b0"], np.float32),
                                   np.asarray(inputs["b1"], np.float32)])[None, :],
            "bm": np.asarray(inputs["bm"], np.float32)[None, :],
            "bo": np.asarray(inputs["bo"], np.float32)[None, :],
            "idx": idx_img, "dstl": dstloc, "dstw": dstw,
            "dstwb": (dstw.astype(_ml.bfloat16) if AGG_BF16 else dstw),
        })
    return in_maps


def _get_program(edge_index):
    if "prog" in _CACHE:
        return _CACHE["prog"]
    pre = _preprocess(edge_index)
    calls, block_tiles, ht_tiles = _build_callplan(pre["tiles_hb"])
    tiles_total = int(pre["tiles_hb"].sum())
    nc = _build_bass(pre, calls, block_tiles, ht_tiles, len(calls) * 32, tiles_total)
    _CACHE["prog"] = (nc, pre, calls)
    return _CACHE["prog"]


LAST_EXEC_NS = None


def kernel(**inputs):
    global LAST_EXEC_NS
    _install_hooks()
    from concourse.bass_utils import run_bass_kernel_spmd

    nc, pre, calls = _get_program(inputs["edge_index"])
    in_maps = _make_inmaps(inputs, pre, calls)
    trace = os.environ.get("KERNEL_TRACE", "0") == "1"
    res = run_bass_kernel_spmd(nc, in_maps, list(range(NCORES)), trace=trace)
    LAST_EXEC_NS = res.exec_time_ns
    return np.concatenate([np.asarray(res.results[c]["out"]) for c in range(NCORES)], axis=0)


# revision 10
# speedup vs baseline: 2.0276x; 1.1333x over previous
"""Self-contained Trainium2 Bass kernel for 4-layer GraphSAGE (nn_LASAGE).

Strategy:
  - Nodes dst-sharded across 8 cores (6250/core, padded to 6272 = 49 blocks of 128).
  - Aggregation is done POST-matmul: agg(x)@Wl == agg(x@Wl), so per layer each
    core computes y = h @ Wl for its own shard, an AllGather replicates the full
    Y table [50176, d] to every core's DRAM, and edges gather y[src] rows with
    dma_gather (int16 idx -> table split in two halves, 4 SWDGE queues).
  - Scatter-add into dst blocks via weighted-one-hot matmuls on the PE:
    psumT[feat, dst] += G[e, feat] contracted with onehot[e, dst], where
    onehot[e, dst] = (iota == dstcol[e]) * invdeg[dst[e]] is built in a single
    DVE tensor_scalar op. The Wr-side and bias (K=1 matmul vs ones) accumulate
    into the same PSUM, so the epilogue is a single fused relu.
  - Layer1 fuses conv0+conv1 (concat -> 256 feat). Layer3 (output, d=64) uses
    non-transposed psum so rows DMA straight to the output.
"""
import sys, os, types

sys.path.insert(0, "/opt/trn_rl_repo")
import numpy as np

N = 50000
E = 800000
NCORES = 8
S = N // NCORES            # 6250 real nodes per core
SP = 6272                  # padded (49 blocks of 128)
NBLK = SP // 128
SPH = SP // 2              # 3136: local-row split for the two AG half-tables
HALF = NCORES * SPH        # 25088 rows per half-table (int16-safe)
D1 = 256                   # concat(h0, h1)
DM = 256
DO = 64
MAXI = 512                 # max idxs per dma_gather call
TPC = MAXI // 128          # tiles per full call
AGG_BF16 = os.environ.get("KERNEL_F32", "0") != "1"   # bf16 gather tables/one-hots


def _install_hooks():
    """antenv.axon_hooks shim so trace=True works in this image (optional)."""
    try:
        import antenv
        if "antenv.axon_hooks" not in sys.modules:
            mod = types.ModuleType("antenv.axon_hooks")
            mod._hook = None
            mod.set_axon_ntff_profile_hook = lambda h: setattr(mod, "_hook", h)
            mod.get_axon_ntff_profile_hook = lambda: mod._hook
            sys.modules["antenv.axon_hooks"] = mod
            antenv.axon_hooks = mod
        from antenv.axon_hooks import get_axon_ntff_profile_hook, set_axon_ntff_profile_hook
        if get_axon_ntff_profile_hook() is None:
            from trn_agent_boot.trn_boot import _ntff_profile_via_ctypes
            set_axon_ntff_profile_hook(_ntff_profile_via_ctypes("/opt/axon/libaxon_pjrt.so"))
        import concourse.bass_utils as bu
        bu.upload_artifacts = lambda tmpdir: f"file://{tmpdir}"
    except Exception:
        pass


def _preprocess(edge_index):
    """Edge lists per core, grouped by (dst block, src half), padded per-tile."""
    src = np.asarray(edge_index[0], np.int64)
    dst = np.asarray(edge_index[1], np.int64)
    core = dst // S
    dl = (dst % S).astype(np.int64)
    blk = dl // 128
    col = dl % 128
    sloc = src % S
    half = (sloc >= SPH).astype(np.int64)
    grow = (src // S) * SPH + (sloc - half * SPH)   # row within its half-table

    deg = np.bincount(core * S + dl, minlength=N).reshape(NCORES, S)

    order = np.lexsort((grow, blk, half, core))
    core_s, half_s, blk_s, col_s, row_s = (core[order], half[order], blk[order],
                                           col[order], grow[order])

    key = (core_s * 2 + half_s) * NBLK + blk_s
    counts = np.bincount(key, minlength=NCORES * 2 * NBLK).reshape(NCORES, 2, NBLK)
    tiles_hb = np.ceil(counts.max(axis=0) / 128).astype(np.int64)   # [2, NBLK]
    tiles_hb = np.maximum(tiles_hb, 1)

    pad_hb = tiles_hb * 128
    tot_h = pad_hb.sum(axis=1)
    seg_off = np.zeros((2, NBLK), np.int64)
    seg_off[:, 1:] = np.cumsum(pad_hb, axis=1)[:, :-1]

    srcpad = np.zeros((NCORES, 2), dtype=object)
    colpad = np.zeros((NCORES, 2), dtype=object)
    for c in range(NCORES):
        for h in range(2):
            srcpad[c, h] = np.zeros(int(tot_h[h]), np.int64)
            colpad[c, h] = np.full(int(tot_h[h]), -1, np.int64)
    grp = key
    first = np.r_[True, grp[1:] != grp[:-1]]
    gidx = np.arange(len(grp)) - np.maximum.accumulate(np.where(first, np.arange(len(grp)), 0))
    pos = seg_off[half_s, blk_s] + gidx
    for c in range(NCORES):
        m = core_s == c
        for h in range(2):
            mh = m & (half_s == h)
            p = pos[mh]
            srcpad[c, h][p] = row_s[mh]
            colpad[c, h][p] = col_s[mh]

    return {
        "tiles_hb": tiles_hb, "seg_off": seg_off,
        "srcpad": srcpad, "colpad": colpad, "deg": deg,
    }


def _build_callplan(tiles_hb):
    """Gather call plan (compile-time, same for every core)."""
    calls = []
    block_tiles = {b: [] for b in range(NBLK)}
    tile_ctr = [0, 0]
    ht_tiles = [int(tiles_hb[0].sum()), int(tiles_hb[1].sum())]
    for b in range(NBLK):
        for h in range(2):
            nt = int(tiles_hb[h, b])
            done = 0
            while done < nt:
                k = min(TPC, nt - done)
                ci = len(calls)
                dcol0 = (0 if h == 0 else ht_tiles[0]) + tile_ctr[h]
                calls.append(dict(h=h, k=k, tile_base=tile_ctr[h], blk=b, dcol0=dcol0))
                for j in range(k):
                    dcol = (0 if h == 0 else ht_tiles[0]) + tile_ctr[h] + j
                    block_tiles[b].append((ci, j, dcol))
                tile_ctr[h] += k
                done += k
    return calls, block_tiles, ht_tiles


def _idx_arrays(pre, calls, core, invd_local):
    """int16 idx image [128, ncalls*32], dstloc & dstw [128, ntiles] f32."""
    ncalls = len(calls)
    idx_img = np.zeros((16, ncalls * 32), np.int16)
    tiles_total = int(pre["tiles_hb"].sum())
    dstloc = np.full((128, tiles_total), -1.0, np.float32)
    dstw = np.zeros((128, tiles_total), np.float32)
    ht0 = int(pre["tiles_hb"][0].sum())
    for ci, cl in enumerate(calls):
        h, k, tb, b = cl["h"], cl["k"], cl["tile_base"], cl["blk"]
        e0 = int(pre["seg_off"][h, b]) + (tb - int(pre["tiles_hb"][h, :b].sum())) * 128
        nidx = k * 128
        seg_src = pre["srcpad"][core, h][e0:e0 + nidx]
        seg_col = pre["colpad"][core, h][e0:e0 + nidx]
        idx_img[:, ci * 32: ci * 32 + (nidx // 16)] = seg_src.reshape(-1, 16).T.astype(np.int16)
        for t in range(k):
            dcol = (0 if h == 0 else ht0) + tb + t
            cc = seg_col[t * 128:(t + 1) * 128]
            dstloc[:, dcol] = cc
            w = np.where(cc >= 0, invd_local[np.clip(b * 128 + cc, 0, S - 1)], 0.0)
            dstw[:, dcol] = w.astype(np.float32)
    return np.tile(idx_img, (8, 1)), dstloc, dstw


def _build_bass(pre, calls, block_tiles, ht_tiles, ncalls_cols, tiles_total):
    import concourse.bass as bass
    import concourse.bacc as bacc
    import concourse.mybir as mybir
    import concourse.tile as tile

    FP32 = mybir.dt.float32
    BF16 = mybir.dt.bfloat16
    YDT = BF16 if AGG_BF16 else FP32
    I16 = mybir.dt.int16
    AL = mybir.AluOpType
    AF = mybir.ActivationFunctionType

    nc = bacc.Bacc("TRN2", target_bir_lowering=False, debug=False,
                   enable_asserts=False, num_devices=NCORES, num_swdge_queues=4)

    x0T = nc.dram_tensor("x0T", [128, SP], FP32, kind="ExternalInput")
    x1T = nc.dram_tensor("x1T", [128, SP], FP32, kind="ExternalInput")
    wl0 = nc.dram_tensor("wl0", [128, 128], FP32, kind="ExternalInput")
    wr0 = nc.dram_tensor("wr0", [128, 128], FP32, kind="ExternalInput")
    wl1 = nc.dram_tensor("wl1", [128, 128], FP32, kind="ExternalInput")
    wr1 = nc.dram_tensor("wr1", [128, 128], FP32, kind="ExternalInput")
    wlm = nc.dram_tensor("wlm", [256, 256], FP32, kind="ExternalInput")
    wrm = nc.dram_tensor("wrm", [256, 256], FP32, kind="ExternalInput")
    wlo = nc.dram_tensor("wlo", [256, 64], FP32, kind="ExternalInput")
    wro = nc.dram_tensor("wro", [256, 64], FP32, kind="ExternalInput")
    b01d = nc.dram_tensor("b01", [1, 256], FP32, kind="ExternalInput")
    bmd = nc.dram_tensor("bm", [1, 256], FP32, kind="ExternalInput")
    bod = nc.dram_tensor("bo", [1, 64], FP32, kind="ExternalInput")
    idxd = nc.dram_tensor("idx", [128, ncalls_cols], I16, kind="ExternalInput")
    dstld = nc.dram_tensor("dstl", [128, tiles_total], FP32, kind="ExternalInput")
    dstwd = nc.dram_tensor("dstw", [128, tiles_total], FP32, kind="ExternalInput")
    dstwbd = nc.dram_tensor("dstwb", [128, tiles_total], YDT, kind="ExternalInput")
    outd = nc.dram_tensor("out", [S, DO], FP32, kind="ExternalOutput")

    with tile.TileContext(nc) as tc:
        with (
            tc.tile_pool(name="const", bufs=1) as cp,
            tc.tile_pool(name="acts", bufs=1) as hp,
            tc.tile_pool(name="g", bufs=10) as gp,
            tc.tile_pool(name="oh", bufs=4) as ohp,
            tc.tile_pool(name="xs", bufs=2) as xsp,
            # PSUM budget (8 banks): ps0(2) + ps1(2) + py(2) = 6 banks
            tc.tile_pool(name="ps", bufs=2, space="PSUM") as psp,
            tc.tile_pool(name="psy", bufs=2, space="PSUM") as psyp,
            tc.tile_pool(name="ev", bufs=2) as evp,
            tc.tile_pool(name="dram", bufs=1, space="DRAM") as dp,
        ):
            def load(name, dt_, shape, src):
                t = cp.tile(shape, dt_, name=name)
                nc.sync.dma_start(out=t[:], in_=src)
                return t

            wl0t = load("wl0t", FP32, [128, 128], wl0[:])
            wr0t = load("wr0t", FP32, [128, 128], wr0[:])
            wl1t = load("wl1t", FP32, [128, 128], wl1[:])
            wr1t = load("wr1t", FP32, [128, 128], wr1[:])
            wlmt = [load(f"wlmt{i}", FP32, [128, 256], wlm[i * 128:(i + 1) * 128, :]) for i in range(2)]
            wrmt = [load(f"wrmt{i}", FP32, [128, 256], wrm[i * 128:(i + 1) * 128, :]) for i in range(2)]
            wlot = [load(f"wlot{i}", FP32, [128, 64], wlo[i * 128:(i + 1) * 128, :]) for i in range(2)]
            wrot = [load(f"wrot{i}", FP32, [128, 64], wro[i * 128:(i + 1) * 128, :]) for i in range(2)]
            b01t = load("b01t", FP32, [1, 256], b01d[:])
            bmt = load("bmt", FP32, [1, 256], bmd[:])
            bot = load("bot", FP32, [1, 64], bod[:])
            idxt = load("idxt", I16, [128, ncalls_cols], idxd[:])
            dstl = load("dstlt", FP32, [128, tiles_total], dstld[:])
            dstw = load("dstwt", FP32, [128, tiles_total], dstwd[:])
            dstwb = load("dstwbt", YDT, [128, tiles_total], dstwbd[:])

            ones_r = cp.tile([1, 128], FP32, name="ones_r")
            nc.vector.memset(ones_r[:], 1.0)
            iota_i = cp.tile([128, TPC, 128], mybir.dt.int32, name="iota_i")
            nc.gpsimd.iota(iota_i[:], pattern=[[0, TPC], [1, 128]], base=0,
                           channel_multiplier=0)
            iota_f = cp.tile([128, TPC, 128], FP32, name="iota_f")
            nc.vector.tensor_copy(out=iota_f[:], in_=iota_i[:])

            hT = [hp.tile([128, SP], FP32, name=f"hT{i}") for i in range(2)]
            h2T = [hp.tile([128, SP], FP32, name=f"h2T{i}") for i in range(2)]

            shared = "Shared" if NCORES > 4 else "Local"
            y01_own = [dp.tile([SPH, D1], YDT, name=f"y01_own{h}") for h in range(2)]
            Y01 = [dp.tile([HALF, D1], YDT, name=f"Y01{h}", addr_space=shared) for h in range(2)]
            ym_own = [dp.tile([SPH, DM], YDT, name=f"ym_own{h}") for h in range(2)]
            Ym = [dp.tile([HALF, DM], YDT, name=f"Ym{h}", addr_space=shared) for h in range(2)]
            yo_own = [dp.tile([SPH, DO], FP32, name=f"yo_own{h}") for h in range(2)]
            Yo = [dp.tile([HALF, DO], FP32, name=f"Yo{h}", addr_space=shared) for h in range(2)]

            def write_y(dsts, b, src_tile, d):
                r0 = b * 128
                if r0 + 128 <= SPH:
                    nc.sync.dma_start(out=dsts[0][r0:r0 + 128, :], in_=src_tile[:])
                elif r0 >= SPH:
                    nc.sync.dma_start(out=dsts[1][r0 - SPH:r0 - SPH + 128, :], in_=src_tile[:])
                else:
                    nlo = SPH - r0
                    nc.sync.dma_start(out=dsts[0][r0:SPH, :], in_=src_tile[0:nlo, :])
                    nc.sync.dma_start(out=dsts[1][0:128 - nlo, :], in_=src_tile[nlo:128, :])

            RG = [list(range(NCORES))]

            def blk_sl(b):
                return slice(b * 128, (b + 1) * 128)

            def onehot_call(cl, dt, wsrc):
                k, d0 = cl["k"], cl["dcol0"]
                oh = ohp.tile([128, TPC, 128], dt, name="oh",
                              tag="oh" if dt == YDT else "ohf",
                              padded_shape=[128, TPC, 128])
                nc.vector.tensor_tensor(
                    out=oh[:, 0:k, :], in0=iota_f[:, 0:k, :],
                    in1=dstl[:, d0:d0 + k].to_broadcast([128, k, 128]),
                    op=AL.is_equal)
                nc.vector.tensor_tensor(
                    out=oh[:, 0:k, :], in0=oh[:, 0:k, :],
                    in1=wsrc[:, d0:d0 + k].to_broadcast([128, k, 128]),
                    op=AL.mult)
                return oh

            # ================= L1 pre: y01_own = [x0@Wl0 | x1@Wl1] =========
            for b in range(NBLK):
                x0b = xsp.tile([128, 128], FP32, name="x0b", tag="x0b")
                nc.sync.dma_start(out=x0b[:], in_=x0T[:, blk_sl(b)])
                x1b = xsp.tile([128, 128], FP32, name="x1b", tag="x1b")
                nc.sync.dma_start(out=x1b[:], in_=x1T[:, blk_sl(b)])
                py0 = psyp.tile([128, 128], FP32, name="py0", tag="py", padded_shape=[128, 256])
                py1 = psyp.tile([128, 128], FP32, name="py1", tag="py", padded_shape=[128, 256])
                nc.tensor.matmul(py0[:], lhsT=x0b[:], rhs=wl0t[:], start=True, stop=True)
                nc.tensor.matmul(py1[:], lhsT=x1b[:], rhs=wl1t[:], start=True, stop=True)
                evy = evp.tile([128, 256], YDT, name="evy", tag="evy")
                nc.vector.tensor_copy(out=evy[:, 0:128], in_=py0[:])
                nc.vector.tensor_copy(out=evy[:, 128:256], in_=py1[:])
                write_y(y01_own, b, evy, D1)
                if b == NBLK // 2:
                    nc.gpsimd.collective_compute(
                        "AllGather", AL.bypass, replica_groups=RG,
                        ins=[y01_own[0][:]], outs=[Y01[0][:]])
            nc.gpsimd.collective_compute(
                "AllGather", AL.bypass, replica_groups=RG,
                ins=[y01_own[1][:]], outs=[Y01[1][:]])

            # ================= aggregation layer (L1/L2) =====================
            def agg_layer(Ytab, wr_tiles, bias_t, h_src, h_dst, wl_next, y_next,
                          d_next, ynext_dt, ag_lo=None, ag_hi=None):
                gtiles = {}
                ohs = {}
                qn = [0]

                def emit_gathers(cis):
                    for ci in cis:
                        cl = calls[ci]
                        k = cl["k"]
                        g = gp.tile([128, TPC, D1], YDT, name="g", tag="g")
                        nc.gpsimd.dma_gather(
                            out_ap=g[:, 0:k, :],
                            in_ap=Ytab[cl["h"]][:],
                            idxs_ap=idxt[:, ci * 32: ci * 32 + (k * 128) // 16],
                            num_idxs=k * 128, num_idxs_reg=k * 128,
                            elem_size=D1, queue_num=qn[0] % 4)
                        qn[0] += 1
                        gtiles[ci] = g
                        ohs[ci] = onehot_call(cl, YDT, dstwb)

                for b in range(NBLK):
                    cis = sorted({ci for ci, _, _ in block_tiles[b]})
                    emit_gathers(cis)
                    ps0 = psp.tile([128, 128], FP32, name="ps0", tag="ps0")
                    ps1 = psp.tile([128, 128], FP32, name="ps1", tag="ps1")
                    if h_src is None:
                        x0b = xsp.tile([128, 128], FP32, name="x0b2", tag="x0b")
                        nc.sync.dma_start(out=x0b[:], in_=x0T[:, blk_sl(b)])
                        x1b = xsp.tile([128, 128], FP32, name="x1b2", tag="x1b")
                        nc.sync.dma_start(out=x1b[:], in_=x1T[:, blk_sl(b)])
                        nc.tensor.matmul(ps0[:], lhsT=wr0t[:], rhs=x0b[:], start=True, stop=False)
                        nc.tensor.matmul(ps1[:], lhsT=wr1t[:], rhs=x1b[:], start=True, stop=False)
                    else:
                        hs = [h_src[0][:, blk_sl(b)], h_src[1][:, blk_sl(b)]]
                        nc.tensor.matmul(ps0[:], lhsT=wr_tiles[0][:, 0:128], rhs=hs[0], start=True, stop=False)
                        nc.tensor.matmul(ps0[:], lhsT=wr_tiles[1][:, 0:128], rhs=hs[1], start=False, stop=False)
                        nc.tensor.matmul(ps1[:], lhsT=wr_tiles[0][:, 128:256], rhs=hs[0], start=True, stop=False)
                        nc.tensor.matmul(ps1[:], lhsT=wr_tiles[1][:, 128:256], rhs=hs[1], start=False, stop=False)
                    nc.tensor.matmul(ps0[:], lhsT=bias_t[0:1, 0:128], rhs=ones_r[0:1, :],
                                     start=False, stop=False)
                    nc.tensor.matmul(ps1[:], lhsT=bias_t[0:1, 128:256], rhs=ones_r[0:1, :],
                                     start=False, stop=False)
                    tl = block_tiles[b]
                    for n, (ci, slot, dcol) in enumerate(tl):
                        g = gtiles[ci]
                        oh = ohs[ci]
                        last = (n == len(tl) - 1)
                        nc.tensor.matmul(ps0[:], lhsT=g[:, slot, 0:128], rhs=oh[:, slot, :],
                                         start=False, stop=last)
                        nc.tensor.matmul(ps1[:], lhsT=g[:, slot, 128:256], rhs=oh[:, slot, :],
                                         start=False, stop=last)
                    nc.scalar.activation(h_dst[0][:, blk_sl(b)], ps0[:], AF.Relu)
                    nc.scalar.activation(h_dst[1][:, blk_sl(b)], ps1[:], AF.Relu)
                    pyn = psyp.tile([128, d_next], FP32, name="pyn", tag="py",
                                    padded_shape=[128, 256])
                    nc.tensor.matmul(pyn[:], lhsT=h_dst[0][:, blk_sl(b)], rhs=wl_next[0][:],
                                     start=True, stop=False)
                    nc.tensor.matmul(pyn[:], lhsT=h_dst[1][:, blk_sl(b)], rhs=wl_next[1][:],
                                     start=False, stop=True)
                    evn = evp.tile([128, d_next], ynext_dt, name="evn", tag="evy",
                                   padded_shape=[128, 256])
                    nc.vector.tensor_copy(out=evn[:], in_=pyn[:])
                    write_y(y_next, b, evn, d_next)
                    if b == NBLK // 2 and ag_lo is not None:
                        ag_lo()
                if ag_hi is not None:
                    ag_hi()

            def make_ag(src, dst):
                def f():
                    nc.gpsimd.collective_compute(
                        "AllGather", AL.bypass, replica_groups=RG,
                        ins=[src[:]], outs=[dst[:]])
                return f

            agg_layer(Y01, None, b01t, None, hT, wlmt, ym_own, DM, YDT,
                      ag_lo=make_ag(ym_own[0], Ym[0]), ag_hi=make_ag(ym_own[1], Ym[1]))
            agg_layer(Ym, wrmt, bmt, hT, h2T, wlot, yo_own, DO, FP32,
                      ag_lo=make_ag(yo_own[0], Yo[0]), ag_hi=make_ag(yo_own[1], Yo[1]))

            # ================= L3: out[node, 64] ============================
            qn3 = 0
            for b in range(NBLK):
                gtiles3 = {}
                ohs3 = {}
                cis = sorted({ci for ci, _, _ in block_tiles[b]})
                for ci in cis:
                    cl = calls[ci]
                    k = cl["k"]
                    g3 = gp.tile([128, TPC, DO], FP32, name="g3", tag="g3")
                    nc.gpsimd.dma_gather(
                        out_ap=g3[:, 0:k, :], in_ap=Yo[cl["h"]][:],
                        idxs_ap=idxt[:, ci * 32: ci * 32 + (k * 128) // 16],
                        num_idxs=k * 128, num_idxs_reg=k * 128,
                        elem_size=DO, queue_num=qn3 % 4)
                    qn3 += 1
                    gtiles3[ci] = g3
                    ohs3[ci] = onehot_call(cl, FP32, dstw)
                ps3 = psp.tile([128, DO], FP32, name="ps3", tag="ps0",
                               padded_shape=[128, 128])
                nc.tensor.matmul(ps3[:], lhsT=h2T[0][:, blk_sl(b)], rhs=wrot[0][:],
                                 start=True, stop=False)
                nc.tensor.matmul(ps3[:], lhsT=h2T[1][:, blk_sl(b)], rhs=wrot[1][:],
                                 start=False, stop=False)
                nc.tensor.matmul(ps3[:], lhsT=ones_r[0:1, :], rhs=bot[0:1, :],
                                 start=False, stop=False)
                tl = block_tiles[b]
                for n, (ci, slot, dcol) in enumerate(tl):
                    g3 = gtiles3[ci]
                    oh = ohs3[ci]
                    nc.tensor.matmul(ps3[:], lhsT=oh[:, slot, :], rhs=g3[:, slot, :],
                                     start=False, stop=(n == len(tl) - 1))
                osb = evp.tile([128, DO], FP32, name="osb", tag="osb")
                nc.vector.tensor_copy(out=osb[:], in_=ps3[:])
                rows = min(128, S - b * 128)
                nc.sync.dma_start(out=outd[b * 128: b * 128 + rows, :],
                                  in_=osb[0:rows, :])

    nc.finalize()
    return nc


_CACHE = {}


def _make_inmaps(inputs, pre, calls):
    import ml_dtypes as _ml
    x0 = np.asarray(inputs["x0"], np.float32)
    x1 = np.asarray(inputs["x1"], np.float32)
    deg = pre["deg"]
    in_maps = []
    for c in range(NCORES):
        invd_local = (1.0 / np.maximum(deg[c], 1.0)).astype(np.float64)
        idx_img, dstloc, dstw = _idx_arrays(pre, calls, c, invd_local)
        x0c = np.zeros((128, SP), np.float32)
        x0c[:, :S] = x0[c * S:(c + 1) * S, :].T
        x1c = np.zeros((128, SP), np.float32)
        x1c[:, :S] = x1[c * S:(c + 1) * S, :].T
        in_maps.append({
            "x0T": x0c, "x1T": x1c,
            "wl0": np.asarray(inputs["Wl0"], np.float32),
            "wr0": np.asarray(inputs["Wr0"], np.float32),
            "wl1": np.asarray(inputs["Wl1"], np.float32),
            "wr1": np.asarray(inputs["Wr1"], np.float32),
            "wlm": np.asarray(inputs["Wlm"], np.float32),
            "wrm": np.asarray(inputs["Wrm"], np.float32),
            "wlo": np.asarray(inputs["Wlo"], np.float32),
            "wro": np.asarray(inputs["Wro"], np.float32),
            "b01": np.concatenate([np.asarray(inputs["b0"], np.float32),
                                   np.asarray(inputs["b1"], np.float32)])[None, :],
            "bm": np.asarray(inputs["bm"], np.float32)[None, :],
            "bo": np.asarray(inputs["bo"], np.float32)[None, :],
            "idx": idx_img, "dstl": dstloc, "dstw": dstw,
            "dstwb": (dstw.astype(_ml.bfloat16) if AGG_BF16 else dstw),
        })
    return in_maps


def _get_program(edge_index):
    if "prog" in _CACHE:
        return _CACHE["prog"]
    pre = _preprocess(edge_index)
    calls, block_tiles, ht_tiles = _build_callplan(pre["tiles_hb"])
    tiles_total = int(pre["tiles_hb"].sum())
    nc = _build_bass(pre, calls, block_tiles, ht_tiles, len(calls) * 32, tiles_total)
    _CACHE["prog"] = (nc, pre, calls)
    return _CACHE["prog"]


LAST_EXEC_NS = None


def kernel(**inputs):
    global LAST_EXEC_NS
    _install_hooks()
    from concourse.bass_utils import run_bass_kernel_spmd

    nc, pre, calls = _get_program(inputs["edge_index"])
    in_maps = _make_inmaps(inputs, pre, calls)
    trace = os.environ.get("KERNEL_TRACE", "0") == "1"
    res = run_bass_kernel_spmd(nc, in_maps, list(range(NCORES)), trace=trace)
    LAST_EXEC_NS = res.exec_time_ns
    return np.concatenate([np.asarray(res.results[c]["out"]) for c in range(NCORES)], axis=0)
